# revision 1
# baseline (speedup 1.0000x reference)
"""Trainium2 Bass kernel for nn_AttentionTwoStream (two-stream Bahdanau attention
with global gating softmax), distributed over 8 NeuronCores.

Sharding: data-parallel over batch B=64 -> 8 batches per core; weights
replicated. One 16-float-per-core AllGather feeds the global beta softmax
(preceded by a warmup AllGather that hides the ncfw pipeline spin-up).

Compute strategy:
 - frames matmul, score dot and attention einsum run on the TensorEngine in
   fp8e4 DoubleRow mode (2 k-rows per PE pass). Weights that land in fp8 are
   prescaled by 16 (or 256 for gate weights) host-side to clear the fp8
   subnormal floor; descales fold into ACT scale params / the beta scalar.
 - The attention einsum uses exp-unnormalized weights (values ~1.0, fp8-safe)
   against a second [t,h]-layout copy of frames; the softmax divide folds into
   the per-batch evacuation op.
 - gate biases ride the Vector evacuation STTs (no bias matmuls); tanh/exp on
   Scalar; fp32 PSUM accumulation everywhere.
 - DMAs: blobbed by dtype/criticality; sync hwdge queue carries the critical
   score-path set first, scalar/gpsimd queues are gated behind fT[1] so the
   frames stream gets the HBM bandwidth first (per-queue bw is the limit).

Self-contained: hardcodes shapes B=64, Tv=512, Tt=64, H=512, 8 cores.
"""

import numpy as np
import ml_dtypes

import concourse.bacc as bacc
import concourse.mybir as mybir
import concourse.tile as tile
from concourse.bass_utils import run_bass_kernel_spmd

NC = 8          # cores
B = 64          # global batch
BL = B // NC    # batches per core = 8
H = 512
Tv = 512
Tt = 64
KT = H // 128   # 4 contraction tiles (and 4 Tv partition tiles)
F32 = mybir.dt.float32
BF16 = mybir.dt.bfloat16
FP8 = mybir.dt.float8e4
NP_BF16 = ml_dtypes.bfloat16
NP_FP8 = ml_dtypes.float8_e4m3

SW = 16.0       # fp8 prescale for score-path weights
SG = 256.0      # fp8 prescale product for gate/logit-path psums
DR = mybir.MatmulPerfMode.DoubleRow


def build_nc():
    nc = bacc.Bacc(
        "TRN2", target_bir_lowering=False, debug=False,
        enable_asserts=False, num_devices=NC, num_swdge_queues=4,
    )

    def inp(name, shape, dt):
        return nc.dram_tensor(name, list(shape), dt, kind="ExternalInput").ap()

    # --- external inputs (per-core shards; layouts match SBUF tiles) ---
    # fp8 early blob: hT8(64) | vavZ8(64) | uav8 (cols)
    f8e = inp("f8e", (128, 128 + KT * H), FP8)
    wav8 = inp("wav8", (128, KT * H), FP8)
    uat8 = inp("uat8", (128, KT * H), FP8)
    # bf16 smalls blob: eye(128) | hTb(KT*BL) | vatT(KT)
    bfs = inp("bfs", (128, 128 + KT * BL + KT), BF16)
    biasr = inp("biasr", (1, 8 * H), BF16)       # scaled bias rows (warmup fodder)
    biasB = inp("biasB", (BL, 7 * H), BF16)      # per-gate bias rows bcast to 8 parts
    blkI = inp("blkI", (BL, BL * Tt), BF16)      # blkI[b, b'*64+t] = (b==b')
    wbB = inp("wbB", (BL, H), F32)               # wb broadcast to 8 partitions
    fT8 = inp("fT8", (BL, 128, KT * Tv), FP8)    # frames^T [b][p][kt*Tv+t]
    fR8 = inp("fR8", (BL, 128, KT * H), FP8)     # frames   [b][p][tt*H+h], t=tt*128+p
    # fp8 text blob: tT8 | wat8
    f8t = inp("f8t", (128, 2 * KT * H), FP8)
    whh_bf = inp("whh_bf", (128, KT * H), BF16)  # Whh (true scale)
    # fp8 late gate blob: wb8(256x) | vbt8 | wqe8 | vbv8 | wve8 (16x)
    f8g = inp("f8g", (128, 5 * KT * H), FP8)
    out_ext = nc.dram_tensor("out", [BL, H], F32, kind="ExternalOutput").ap()

    ACT = mybir.ActivationFunctionType
    ALU = mybir.AluOpType

    with tile.TileContext(nc) as tc:
        with (
            tc.tile_pool(name="wres", bufs=1) as wres,
            tc.tile_pool(name="work", bufs=4) as work,
            tc.tile_pool(name="small", bufs=1) as small,
            tc.tile_pool(name="psX", bufs=3, space="PSUM") as psX,
            tc.tile_pool(name="psS", bufs=2, space="PSUM") as psS,
            tc.tile_pool(name="psB", bufs=1, space="PSUM") as psB,
            tc.tile_pool(name="psE", bufs=1, space="PSUM") as psE,
            tc.tile_pool(name="psG", bufs=1, space="PSUM") as psG,
            tc.tile_pool(name="dram", bufs=1, space="DRAM") as dram,
        ):
            def load(pool, ap_in, shape, tag, dt, engine, name=None):
                t = pool.tile(list(shape), dt, tag=tag, name=name or tag)
                engine.dma_start(t[:], ap_in)
                return t

            # ---------- warmup collective (primes ncfw; result unused) ----
            warm_out = dram.tile([2 * B, 1], F32, tag="warmout", addr_space="Shared")
            warm_in = dram.tile([2 * BL, 1], F32, tag="warmin")
            nc.gpsimd.collective_compute(
                "AllGather", ALU.bypass,
                replica_groups=[list(range(NC))],
                ins=[warm_in[:].opt()],
                outs=[warm_out[:].opt()],
            )

            # ---------- DMAs ----------
            # pair0 critical set spread across all three queues in parallel:
            # gpsimd pulls fT0/fT1 (4 swdge sub-queues), scalar pulls the
            # P1 fp8 blob, sync pulls wav8 + smalls. Bulk streams follow,
            # gated behind fT1 where they would steal critical bandwidth.
            # sync hwdge:
            biasr_sb = load(wres, biasr, (1, 8 * H), "biasr", BF16, nc.sync)
            wav8_t = load(wres, wav8, (128, KT, H), "wav8", FP8, nc.sync)
            wav8_sb = wav8_t[:, :, :]
            bfs_sb = load(wres, bfs, (128, 128 + KT * BL + KT), "bfs", BF16, nc.sync)
            eye_sb = bfs_sb[:, 0:128]
            hTb_sb = bfs_sb[:, 128:128 + KT * BL].rearrange("p (k b) -> p k b", k=KT)
            vatT_sb = bfs_sb[:, 128 + KT * BL:]
            blkI_sb = load(wres, blkI, (BL, BL * Tt), "blkI", BF16, nc.sync)
            biasB_sb = load(wres, biasB, (BL, 7 * H), "biasB", BF16, nc.sync)
            fT_sb = [None] * BL
            for b in range(2, BL):
                t = wres.tile([128, KT, Tv], FP8, tag=f"fT{b}", name=f"fTs{b}")
                nc.sync.dma_start(t[:], fT8[b])
                fT_sb[b] = t
            wbB_sb = load(wres, wbB, (BL, H), "wbB", F32, nc.sync)
            # gpsimd swdge: f8e + fT0/fT1 first (parallel sub-queues), gated bulk
            f8e_sb = load(wres, f8e, (128, 128 + KT * H), "f8e", FP8, nc.gpsimd)
            hT8_sb = f8e_sb[:, 0:64].rearrange("p (k s) -> p k s", k=KT)
            vavZ8_sb = f8e_sb[:, 64:128].rearrange("p (a b c) -> p a b c", a=2, b=2)
            uav8_sb = f8e_sb[:, 128:].rearrange("p (k n) -> p k n", k=KT)
            for b in range(2):
                t = wres.tile([128, KT, Tv], FP8, tag=f"fT{b}", name=f"fTs{b}")
                nc.gpsimd.dma_start(t[:], fT8[b])
                fT_sb[b] = t
            qgate_g = small.tile([1, 2], FP8, tag="qgate_g")
            nc.gpsimd.tensor_copy(qgate_g[:], fT_sb[1][0:1, 0, 0:2])
            fR_sb = [None] * BL
            for b in range(4):
                t = wres.tile([128, KT, H], FP8, tag=f"fR{b}", name=f"fRs{b}")
                nc.gpsimd.dma_start(t[:], fR8[b])
                fR_sb[b] = t
            f8g_sb = load(wres, f8g, (128, 5 * KT * H), "f8g", FP8, nc.gpsimd)
            wb8_sb = f8g_sb[:, 0:KT * H].rearrange("p (k n) -> p k n", k=KT)
            vbt8_sb = f8g_sb[:, KT * H:2 * KT * H].rearrange("p (k n) -> p k n", k=KT)
            wqe8_sb = f8g_sb[:, 2 * KT * H:3 * KT * H].rearrange("p (k n) -> p k n", k=KT)
            for b in range(4, BL):
                t = wres.tile([128, KT, H], FP8, tag=f"fR{b}", name=f"fRs{b}")
                nc.gpsimd.dma_start(t[:], fR8[b])
                fR_sb[b] = t
            vbv8_sb = f8g_sb[:, 3 * KT * H:4 * KT * H].rearrange("p (k n) -> p k n", k=KT)
            wve8_sb = f8g_sb[:, 4 * KT * H:].rearrange("p (k n) -> p k n", k=KT)
            # scalar hwdge: uat8 first, gated text/late weights
            uat8_t = load(wres, uat8, (128, KT, H), "uat8", FP8, nc.scalar)
            uat8_sb = uat8_t[:, :, :]
            qgate_s = small.tile([1, 2], FP8, tag="qgate_s")
            nc.scalar.copy(qgate_s[:], fT_sb[1][0:1, 0, 0:2])
            f8t_sb = load(wres, f8t, (128, 2 * KT * H), "f8t", FP8, nc.scalar)
            tT8_sb = f8t_sb[:, 0:KT * H].rearrange("p (k n) -> p k n", k=KT)
            wat8_sb = f8t_sb[:, KT * H:].rearrange("p (k n) -> p k n", k=KT)
            whh_sb = load(wres, whh_bf, (128, KT, H), "whh", BF16, nc.scalar)

            ones_sb = small.tile([1, 128], BF16, tag="ones")
            nc.vector.memset(ones_sb[:], 1.0)
            oq_sb = small.tile([1, BL], BF16, tag="oq")
            nc.vector.memset(oq_sb[:], 1.0 / SG)
            onesC_sb = small.tile([128, 1], BF16, tag="onesC")
            nc.vector.memset(onesC_sb[:], 1.0)

            def bB(i):
                return biasB_sb[:, i * H:(i + 1) * H]
            # cols: 16bav, 16bat, 256bve, 256bqe, 256bbv, 256bbt, bh
            bavB, batB, bveB, bqeB, bbvB, bbtB, bhB = (bB(i) for i in range(7))

            # PE warmup: junk matmuls on the first-arriving bytes (p-state
            # ramp while the frames stream loads)
            warm_ps = psX.tile([128, Tv], F32, tag="psX", name="warmps")
            for w in range(6):
                nc.tensor.matmul(
                    warm_ps[0:BL, :], ones_sb[0:1, 0:BL], biasr_sb[0:1, 0:H],
                    start=True, stop=True, skip_group_check=True,
                )

            # fp8 DoubleRow gate: psum[8,H] += (hT|hvT|htT).T @ W  (K=512 via
            # 2 DoubleRow passes), optional extra rows merged by caller.
            def dr_gate(ps, lhs_sb, w_sb, start, stop):
                for ktp in range(2):
                    nc.tensor.matmul(
                        ps[:], lhs_sb[:, 2 * ktp:2 * ktp + 2, 0:BL],
                        w_sb[:, 2 * ktp:2 * ktp + 2, :],
                        start=(start and ktp == 0), stop=(stop and ktp == 1),
                        perf_mode=DR, skip_group_check=True,
                    )

            # ---------- P1: h-projections (score biases) ----------
            uhvb_ps = psG.tile([BL, H], F32, tag="psG", name="uhvb")
            dr_gate(uhvb_ps, hT8_sb, uav8_sb, True, True)
            uhvb_s = small.tile([BL, H], BF16, tag="uhvb_s")
            # uhvb = (16*hUav)/16 + bav
            nc.vector.scalar_tensor_tensor(
                out=uhvb_s[:], in0=uhvb_ps[:], scalar=1.0 / SW, in1=bavB,
                op0=ALU.mult, op1=ALU.add,
            )

            # frames bias in [512,8] layout for per-partition ACT bias
            uhvbT_sb = small.tile([128, KT * BL], F32, tag="uhvbT")
            for jt in range(KT):
                tp = psB.tile([128, BL], BF16, tag="psB", name=f"tpv{jt}")
                nc.tensor.transpose(
                    tp[:], uhvb_s[0:BL, jt * 128:(jt + 1) * 128],
                    eye_sb[0:BL, 0:BL],
                )
                nc.vector.tensor_copy(uhvbT_sb[:, jt * BL:(jt + 1) * BL], tp[:])

            uhtb_ps = psG.tile([BL, H], F32, tag="psG", name="uhtb")
            dr_gate(uhtb_ps, hT8_sb, uat8_sb, True, True)
            uhtb_s = small.tile([BL, H], BF16, tag="uhtb_s")   # 16x scale
            nc.vector.scalar_tensor_tensor(
                out=uhtb_s[:], in0=uhtb_ps[:], scalar=1.0, in1=batB,
                op0=ALU.mult, op1=ALU.add,
            )

            # ---------- frames pair machinery ----------
            NP = BL // 2
            hv16 = small.tile([BL, H], BF16, tag="hv16")
            hv16n = small.tile([BL, H], BF16, tag="hv16n")
            sum8 = small.tile([BL, 1], F32, tag="sum8")
            hvT16 = small.tile([128, KT, 16], FP8, tag="hvT16")
            yv_tiles = {}
            scv_tiles = {}
            avT_tiles = {}
            hv16row = small.tile([1, BL * H], BF16, tag="hv16row")

            def pair_compute(g):
                """xps matmuls + tanh + score accumulation for pair g."""
                scv_g = psS.tile([2, Tv], F32, tag="scS", name=f"scv{g}")
                scv_tiles[g] = scv_g
                nmm = 0
                for jtp in range(2):
                    for i in range(2):
                        b = 2 * g + i
                        yv = work.tile([128, 2, Tv], FP8, tag="yv",
                                       name=f"yv{g}_{jtp}_{i}", bufs=6)
                        for q in range(2):
                            jt = 2 * jtp + q
                            xps = psX.tile([128, Tv], F32, tag="psX",
                                           name=f"xps{g}_{jtp}_{i}_{q}")
                            for ktp in range(2):
                                nc.tensor.matmul(
                                    xps[:],
                                    wav8_sb[:, 2 * ktp:2 * ktp + 2,
                                            jt * 128:(jt + 1) * 128],
                                    fT_sb[b][:, 2 * ktp:2 * ktp + 2, :],
                                    start=(ktp == 0), stop=(ktp == 1),
                                    perf_mode=DR, skip_group_check=True,
                                )
                            # yv = tanh(xps/16 + Uhv + bav)
                            nc.scalar.activation(
                                yv[:, q, :], xps[:], ACT.Tanh,
                                scale=1.0 / SW,
                                bias=uhvbT_sb[:, jt * BL + b: jt * BL + b + 1],
                            )
                        yv_tiles[(g, jtp, i)] = yv
                        nmm += 1
                        nc.tensor.matmul(    # scv += (16Vav).T @ yv -> 16*s
                            scv_g[:],
                            vavZ8_sb[:, jtp, :, 2 * i:2 * i + 2],
                            yv[:, :, :],
                            start=(nmm == 1), stop=(nmm == 4),
                            perf_mode=DR, skip_group_check=True,
                        )

            def pair_chain(g):
                """exp -> transpose weights -> PE einsum -> evac for pair g.
                Normalization is deferred: evacs write 16*unnormalized rows;
                sumv is shipped to sum8 for one post-assembly divide."""
                last = (g == NP - 1)
                tpool, epool = (psX, psX) if last else (psB, psE)
                scv_g = scv_tiles[g]
                avp = small.tile([2, Tv], BF16, tag="avp", name=f"avp{g}", bufs=2)
                sumv = small.tile([2, 1], F32, tag="sumv", name=f"sumv{g}", bufs=2)
                nc.scalar.activation(
                    avp[:], scv_g[:], ACT.Exp, scale=1.0 / SW,
                    accum_out=sumv[:],
                )
                nc.sync.dma_start(sum8[2 * g:2 * g + 2, 0:1], sumv[:, :])
                avT = small.tile([128, KT, 16], FP8, tag="avT", name=f"avT{g}", bufs=2)
                avT_tiles[g] = avT
                for tt in range(KT):
                    tp = tpool.tile([128, 2], BF16, tag=tpool.name, name=f"avtp{g}_{tt}")
                    nc.tensor.transpose(
                        tp[:], avp[0:2, tt * 128:(tt + 1) * 128],
                        eye_sb[0:2, 0:2],
                    )
                    nc.vector.tensor_copy(avT[:, tt, 0:2], tp[:])
                for i in range(2):
                    b = 2 * g + i
                    eps = epool.tile([1, H], F32, tag=epool.name, name=f"eps{g}_{i}")
                    for ttp in range(2):
                        nc.tensor.matmul(
                            eps[:],
                            avT[:, 2 * ttp:2 * ttp + 2, i:i + 1],
                            fR_sb[b][:, 2 * ttp:2 * ttp + 2, :],
                            start=(ttp == 0), stop=(ttp == 1),
                            perf_mode=DR, skip_group_check=True,
                        )
                    # hv16row[b] = 16 * unnormalized weighted sum
                    nc.vector.tensor_scalar_mul(
                        hv16row[0:1, b * H:(b + 1) * H], eps[:], SW,
                    )
                nc.sync.dma_start(
                    hv16[2 * g:2 * g + 2, :],
                    hv16row[0:1, 2 * g * H:(2 * g + 2) * H],
                )

            # --- pair 3 split per-batch: batch 6's chain overlaps batch 7 ---
            scv3 = {}
            avT3 = small.tile([128, KT, 16], FP8, tag="avT", name="avT3", bufs=2)

            def pair3_batch(i):
                b = 6 + i
                scv_b = psS.tile([1, Tv], F32, tag="scS", name=f"scv3_{i}")
                scv3[i] = scv_b
                nmm = 0
                for jtp in range(2):
                    yv = work.tile([128, 2, Tv], FP8, tag="yv",
                                   name=f"yv3_{jtp}_{i}", bufs=6)
                    for q in range(2):
                        jt = 2 * jtp + q
                        xps = psX.tile([128, Tv], F32, tag="psX",
                                       name=f"xps3_{jtp}_{i}_{q}")
                        for ktp in range(2):
                            nc.tensor.matmul(
                                xps[:],
                                wav8_sb[:, 2 * ktp:2 * ktp + 2,
                                        jt * 128:(jt + 1) * 128],
                                fT_sb[b][:, 2 * ktp:2 * ktp + 2, :],
                                start=(ktp == 0), stop=(ktp == 1),
                                perf_mode=DR, skip_group_check=True,
                            )
                        nc.scalar.activation(
                            yv[:, q, :], xps[:], ACT.Tanh,
                            scale=1.0 / SW,
                            bias=uhvbT_sb[:, jt * BL + b: jt * BL + b + 1],
                        )
                    nmm += 1
                    nc.tensor.matmul(   # single-batch score row
                        scv_b[:],
                        vavZ8_sb[:, jtp, :, 0:1],
                        yv[:, :, :],
                        start=(nmm == 1), stop=(nmm == 2),
                        perf_mode=DR, skip_group_check=True,
                    )

            def chain3_i(i):
                # i==0 runs concurrently with batch 7's xps matmuls -> keep it
                # off the psX banks; i==1 runs after all xps -> psX is idle.
                tpool, epool = (psX, psX) if i == 1 else (psB, psE)
                b = 6 + i
                avp = small.tile([1, Tv], BF16, tag="avp", name=f"avp3_{i}", bufs=2)
                sumv = small.tile([1, 1], F32, tag="sumv", name=f"sumv3_{i}", bufs=2)
                nc.scalar.activation(
                    avp[:], scv3[i][:], ACT.Exp, scale=1.0 / SW,
                    accum_out=sumv[:],
                )
                nc.sync.dma_start(sum8[b:b + 1, 0:1], sumv[:, :])
                for tt in range(KT):
                    tp = tpool.tile([128, 1], BF16, tag=tpool.name, name=f"avtp3_{i}_{tt}")
                    nc.tensor.transpose(
                        tp[:], avp[0:1, tt * 128:(tt + 1) * 128],
                        eye_sb[0:1, 0:1],
                    )
                    nc.vector.tensor_copy(avT3[:, tt, i:i + 1], tp[:])
                eps = epool.tile([1, H], F32, tag=epool.name, name=f"eps3_{i}")
                for ttp in range(2):
                    nc.tensor.matmul(
                        eps[:],
                        avT3[:, 2 * ttp:2 * ttp + 2, i:i + 1],
                        fR_sb[b][:, 2 * ttp:2 * ttp + 2, :],
                        start=(ttp == 0), stop=(ttp == 1),
                        perf_mode=DR, skip_group_check=True,
                    )
                nc.vector.tensor_scalar_mul(
                    hv16row[0:1, b * H:(b + 1) * H], eps[:], SW,
                )
                nc.sync.dma_start(hv16[b:b + 1, :], hv16row[0:1, b * H:(b + 1) * H])

            # ---------- text stream pieces ----------
            def text_matmuls():
                sct_ps = psG.tile([1, BL * Tt], F32, tag="psG", name="sct")
                pend = []

                def flush_sct():
                    for yt_, jt_ in pend:
                        nc.tensor.matmul(
                            sct_ps[:], vatT_sb[:, jt_: jt_ + 1], yt_[:],
                            start=(jt_ == 0), stop=(jt_ == KT - 1),
                            skip_group_check=True,
                        )
                    pend.clear()

                for jt in range(KT):
                    xt_ps = psX.tile([128, BL * Tt], F32, tag="psX", name=f"xt{jt}")
                    for ktp in range(2):
                        nc.tensor.matmul(
                            xt_ps[:],
                            wat8_sb[:, 2 * ktp:2 * ktp + 2, jt * 128:(jt + 1) * 128],
                            tT8_sb[:, 2 * ktp:2 * ktp + 2, :],
                            start=(ktp == 0), stop=False,
                            perf_mode=DR, skip_group_check=True,
                        )
                    nc.tensor.matmul(   # bias: += 16*Uhtb[b, jt*128+j] via blkI
                        xt_ps[:], uhtb_s[0:BL, jt * 128:(jt + 1) * 128], blkI_sb[:],
                        start=False, stop=True, skip_group_check=True,
                    )
                    flush_sct()
                    yt = work.tile([128, BL * Tt], BF16, tag="yt", name=f"yt{jt}")
                    nc.scalar.activation(yt[:], xt_ps[:], ACT.Tanh, scale=1.0 / SW)
                    pend.append((yt, jt))
                flush_sct()
                return sct_ps

            def text_softmax(sct_ps):
                sct_sb = small.tile([1, BL * Tt], F32, tag="sct_sb")
                nc.vector.tensor_copy(sct_sb[:], sct_ps[:])
                st8 = small.tile([BL, Tt], F32, tag="st8")
                nc.sync.dma_start(st8[:, :], sct_sb[0:1, :])
                expt = small.tile([BL, Tt], F32, tag="expt")
                sumt = small.tile([BL, 1], F32, tag="sumt")
                nc.scalar.activation(expt[:], st8[:], ACT.Exp, accum_out=sumt[:])
                rt = small.tile([BL, 1], F32, tag="rt")
                nc.vector.reciprocal(rt[:], sumt[:])
                at_sb = small.tile([BL, Tt], BF16, tag="at")
                nc.vector.tensor_scalar_mul(at_sb[:], expt[:], rt[:])
                atRows = small.tile([1, BL * Tt], BF16, tag="atRows")
                nc.sync.dma_start(atRows[0:1, :], at_sb[:, :])
                atB = []
                for b in range(BL):
                    atB_ps = psB.tile([128, Tt], F32, tag="psB", name=f"atB{b}")
                    src = at_sb[0:1, :] if b == 0 else atRows[0:1, b * Tt:(b + 1) * Tt]
                    nc.tensor.matmul(
                        atB_ps[:], ones_sb[0:1, 0:128], src,
                        start=True, stop=True,
                    )
                    t = work.tile([128, Tt], BF16, tag="atB_sb", name=f"atBs{b}", bufs=8)
                    nc.vector.tensor_copy(t[:], atB_ps[:])
                    atB.append(t)
                return atB

            htT_sb = small.tile([128, KT, BL], F32, tag="htT")

            def text_einsum(atB, kts):
                for kt in kts:
                    for b in range(BL):
                        scrt = work.tile([128, Tt], BF16, tag="scrt")
                        nc.vector.scalar_tensor_tensor(
                            out=scrt[:],
                            in0=tT8_sb[:, kt, b * Tt:(b + 1) * Tt],
                            scalar=1.0,
                            in1=atB[b][:],
                            op0=ALU.mult, op1=ALU.mult,
                            accum_out=htT_sb[:, kt, b:b + 1],
                        )

            # ---------- issue order (drives per-engine schedules) ----------
            pair_compute(0)
            sct_ps = text_matmuls()
            pair_compute(1)
            atB = text_softmax(sct_ps)
            pair_chain(0)
            pair_compute(2)
            text_einsum(atB, [0, 1])

            # wbs = 256*(h@Wb); hwhh = h@Whh + bh (true scale)
            wbs_ps = psG.tile([BL, H], F32, tag="psG", name="wbs")
            dr_gate(wbs_ps, hT8_sb, wb8_sb, True, True)
            wbst_sb = small.tile([BL, H], BF16, tag="wbst_sb")  # 256(hWb+bbt)
            nc.vector.scalar_tensor_tensor(
                out=wbst_sb[:], in0=wbs_ps[:], scalar=1.0, in1=bbtB,
                op0=ALU.mult, op1=ALU.add,
            )
            wbsv_sb = small.tile([BL, H], BF16, tag="wbsv_sb")  # 256(hWb+bbv)
            nc.vector.scalar_tensor_tensor(
                out=wbsv_sb[:], in0=wbs_ps[:], scalar=1.0, in1=bbvB,
                op0=ALU.mult, op1=ALU.add,
            )

            pair_chain(1)
            text_einsum(atB, [2, 3])
            pair_chain(2)
            pair3_batch(0)

            hwhh_ps = psG.tile([BL, H], F32, tag="psG", name="hwhh")
            for kt in range(KT):
                nc.tensor.matmul(
                    hwhh_ps[:], hTb_sb[:, kt, :], whh_sb[:, kt, :],
                    start=(kt == 0), stop=(kt == KT - 1), skip_group_check=True,
                )
            hwhh_sb = small.tile([BL, H], F32, tag="hwhh_sb")
            nc.vector.scalar_tensor_tensor(
                out=hwhh_sb[:], in0=hwhh_ps[:], scalar=1.0, in1=bhB,
                op0=ALU.mult, op1=ALU.add,
            )

            chain3_i(0)
            pair3_batch(1)

            # ---------- text gates (256x psums) ----------
            htT16 = small.tile([128, KT, 16], FP8, tag="htT16")
            nc.vector.tensor_scalar_mul(
                htT16[:, :, 0:BL], htT_sb[:, :, :], SW,
            )
            mt1_ps = psG.tile([BL, H], F32, tag="psG", name="mt1")
            nc.tensor.matmul(   # += 256*(h@Wb + bbt) via identity
                mt1_ps[:], eye_sb[0:BL, 0:BL], wbst_sb[:],
                start=True, stop=False, skip_group_check=True,
            )
            dr_gate(mt1_ps, htT16, vbt8_sb, False, True)
            mtv_t = small.tile([BL, H], F32, tag="mtv_t")
            nc.scalar.activation(mtv_t[:], mt1_ps[:], ACT.Tanh, scale=1.0 / SG)
            lgt_t = small.tile([BL, 1], F32, tag="lgt_t")
            scr8t = small.tile([BL, H], F32, tag="scr8t")
            nc.vector.scalar_tensor_tensor(
                out=scr8t[:], in0=mtv_t[:], scalar=1.0, in1=wbB_sb[:],
                op0=ALU.mult, op1=ALU.mult, accum_out=lgt_t[:],
            )
            ht2_ps = psG.tile([BL, H], F32, tag="psG", name="ht2")
            dr_gate(ht2_ps, htT16, wqe8_sb, True, True)
            ht2_sb = small.tile([BL, H], F32, tag="ht2_sb")
            nc.vector.scalar_tensor_tensor(
                out=ht2_sb[:], in0=ht2_ps[:], scalar=1.0, in1=bqeB,
                op0=ALU.mult, op1=ALU.add,
            )

            chain3_i(1)

            # ---------- frames gates + logits ----------
            rv8 = small.tile([BL, 1], F32, tag="rv8")
            nc.vector.reciprocal(rv8[:], sum8[:])
            nc.vector.tensor_scalar_mul(hv16n[:], hv16[:], rv8[:])
            for jt in range(KT):
                tp = psX.tile([128, BL], BF16, tag="psX", name=f"hvtp{jt}")
                nc.tensor.transpose(
                    tp[:], hv16n[0:BL, jt * 128:(jt + 1) * 128],
                    eye_sb[0:BL, 0:BL],
                )
                nc.vector.tensor_copy(hvT16[:, jt, 0:BL], tp[:])

            mv1_ps = psG.tile([BL, H], F32, tag="psG", name="mv1")
            nc.tensor.matmul(
                mv1_ps[:], eye_sb[0:BL, 0:BL], wbsv_sb[:],
                start=True, stop=False, skip_group_check=True,
            )
            dr_gate(mv1_ps, hvT16, vbv8_sb, False, True)
            mtv_v = small.tile([BL, H], F32, tag="mtv_v")
            nc.scalar.activation(mtv_v[:], mv1_ps[:], ACT.Tanh, scale=1.0 / SG)
            lgv_t = small.tile([BL, 1], F32, tag="lgv_t")
            scr8v = small.tile([BL, H], F32, tag="scr8v")
            nc.vector.scalar_tensor_tensor(
                out=scr8v[:], in0=mtv_v[:], scalar=1.0, in1=wbB_sb[:],
                op0=ALU.mult, op1=ALU.mult, accum_out=lgv_t[:],
            )

            # ---------- AllGather of the 16 local logits ----------
            cc_in = dram.tile([2 * BL, 1], F32, tag="ccin")
            cc_out = dram.tile([2 * B, 1], F32, tag="ccout", addr_space="Shared")
            nc.sync.dma_start(cc_in[0:BL], lgv_t[:])
            nc.sync.dma_start(cc_in[BL:2 * BL], lgt_t[:])
            nc.gpsimd.collective_compute(
                "AllGather", ALU.bypass,
                replica_groups=[list(range(NC))],
                ins=[cc_in[:].opt()],
                outs=[cc_out[:].opt()],
            )

            # overlap the AG: hv2 = 256*(hv@Wve.T + bve)
            hv2_ps = psG.tile([BL, H], F32, tag="psG", name="hv2")
            dr_gate(hv2_ps, hvT16, wve8_sb, True, True)
            hv2_sb = small.tile([BL, H], F32, tag="hv2_sb")
            nc.vector.scalar_tensor_tensor(
                out=hv2_sb[:], in0=hv2_ps[:], scalar=1.0, in1=bveB,
                op0=ALU.mult, op1=ALU.add,
            )

            # ---------- global beta softmax (logits tiny; no max-shift) ----
            g_col = small.tile([2 * B, 1], F32, tag="g")
            nc.sync.dma_start(g_col[:, :], cc_out[:, :])
            ge_col = small.tile([2 * B, 1], BF16, tag="ge")
            nc.scalar.activation(ge_col[:], g_col[:], ACT.Exp)
            z_ps = psE.tile([1, 1], F32, tag="psE", name="zps")
            nc.tensor.matmul(   # Z = sum over partitions
                z_ps[:], onesC_sb[0:128, 0:1], ge_col[0:128, :],
                start=True, stop=True, skip_group_check=True,
            )
            geR_ps = psB.tile([1, 2], BF16, tag="psB", name="geR")
            nc.tensor.transpose(geR_ps[:], ge_col[0:2, 0:1], eye_sb[0:2, 0:2])
            rg = small.tile([1, 1], F32, tag="rg")
            nc.vector.reciprocal(rg[:], z_ps[:])
            betas = small.tile([1, 2], BF16, tag="betas")
            nc.vector.tensor_scalar_mul(betas[:], geR_ps[:], rg[:])
            beta8_ps = psB.tile([BL, 2], F32, tag="psB", name="beta8")
            nc.tensor.matmul(   # beta/256 broadcast to 8 partitions
                beta8_ps[:], oq_sb[0:1, 0:BL], betas[0:1, 0:2],
                start=True, stop=True,
            )
            # ---------- out = tanh(hWhh+bh + b0*hv2 + b1*ht2) ----------
            t1 = small.tile([BL, H], F32, tag="t1")
            nc.vector.scalar_tensor_tensor(
                out=t1[:], in0=hv2_sb[:], scalar=beta8_ps[:, 0:1], in1=hwhh_sb[:],
                op0=ALU.mult, op1=ALU.add,
            )
            s1 = small.tile([BL, H], F32, tag="s1")
            nc.vector.scalar_tensor_tensor(
                out=s1[:], in0=ht2_sb[:], scalar=beta8_ps[:, 1:2], in1=t1[:],
                op0=ALU.mult, op1=ALU.add,
            )
            out_sb = small.tile([BL, H], F32, tag="out_sb")
            nc.scalar.activation(out_sb[:], s1[:], ACT.Tanh)
            nc.sync.dma_start(out_ext, out_sb[:])

    nc.compile()
    return nc


_cached_nc = None


def _get_nc():
    global _cached_nc
    if _cached_nc is None:
        _cached_nc = build_nc()
    return _cached_nc


def _bf(a):
    return np.asarray(a, np.float32).astype(NP_BF16)


def _f8(a):
    return np.asarray(a, np.float32).astype(NP_FP8)


def _pack_w(w):
    """[512,512] -> [128, 4*512] with free = kt*512 + j, k = kt*128 + p."""
    return np.ascontiguousarray(
        np.asarray(w, np.float32).reshape(KT, 128, H).transpose(1, 0, 2)
        .reshape(128, KT * H)
    )


def make_in_maps(inputs):
    h = np.asarray(inputs["h"], np.float32)
    frames = np.asarray(inputs["hidden_frames"], np.float32)
    text = np.asarray(inputs["hidden_text"], np.float32)

    Vav = np.asarray(inputs["Vav"], np.float32)
    Vat = np.asarray(inputs["Vat"], np.float32)
    wb = np.asarray(inputs["wb"], np.float32)

    # vavZ8[p, jtp, q, 2v+c] = 16*Vav[(2jtp+q)*128+p] if c == v (pad to 16)
    vavZ = np.zeros((128, 2, 2, 16), np.float32)
    for jtp in range(2):
        for q in range(2):
            col = Vav[(2 * jtp + q) * 128:(2 * jtp + q + 1) * 128] * SW
            for v in range(2):
                vavZ[:, jtp, q, 2 * v + v] = col

    biasr = np.zeros((1, 8 * H), np.float32)
    biasr = _bf(biasr)
    biasB = np.zeros((BL, 7 * H), np.float32)
    scales = [SW, SW, SG, SG, SG, SG, 1.0]
    for i, (k, s) in enumerate(zip(
            ["bav", "bat", "bve", "bqe", "bbv", "bbt", "bh"], scales)):
        biasB[:, i * H:(i + 1) * H] = np.asarray(inputs[k], np.float32)[None, :] * s
    biasB = _bf(biasB)
    wbB = np.ascontiguousarray(np.broadcast_to(wb, (BL, H))).astype(np.float32)
    blkI = np.zeros((BL, BL, Tt), np.float32)
    for b in range(BL):
        blkI[b, b, :] = 1.0
    blkI = _bf(blkI.reshape(BL, BL * Tt))

    hTp = np.zeros((128, KT, 16), np.float32)   # per-core filled below
    f8e_shared = np.zeros((128, 128 + KT * H), np.float32)
    f8e_shared[:, 64:128] = vavZ.reshape(128, 64)
    f8e_shared[:, 128:] = _pack_w(inputs["Uav"]) * SW
    wav8 = _f8(_pack_w(inputs["Wav"]) * SW)
    uat8 = _f8(_pack_w(inputs["Uat"]) * SW)

    f8t = np.zeros((128, 2 * KT * H), np.float32)
    # tT8 filled per-core below
    f8t[:, KT * H:] = _pack_w(inputs["Wat"]) * SW

    f8g = np.zeros((128, 5 * KT * H), np.float32)
    f8g[:, 0:KT * H] = _pack_w(inputs["Wb"]) * SG
    f8g[:, KT * H:2 * KT * H] = _pack_w(inputs["Vbt"]) * SW
    f8g[:, 2 * KT * H:3 * KT * H] = _pack_w(np.asarray(inputs["Wqe"], np.float32).T) * SW
    f8g[:, 3 * KT * H:4 * KT * H] = _pack_w(inputs["Vbv"]) * SW
    f8g[:, 4 * KT * H:] = _pack_w(np.asarray(inputs["Wve"], np.float32).T) * SW
    f8g = _f8(f8g)

    shared = dict(
        f8g=f8g, wav8=wav8, uat8=uat8,
        whh_bf=_bf(_pack_w(inputs["Whh"])),
        biasr=biasr, biasB=biasB, wbB=wbB, blkI=blkI,
    )

    in_maps = []
    for i in range(NC):
        sl = slice(i * BL, (i + 1) * BL)
        fTc = np.ascontiguousarray(
            frames[sl].transpose(0, 2, 1)       # [BL, H, Tv]
            .reshape(BL, KT, 128, Tv)
            .transpose(0, 2, 1, 3)              # [BL, 128, KT, Tv]
            .reshape(BL, 128, KT * Tv)
        )
        fRc = np.ascontiguousarray(
            frames[sl]                          # [BL, Tv, H]
            .reshape(BL, KT, 128, H)
            .transpose(0, 2, 1, 3)              # [BL, 128, TT, H]
            .reshape(BL, 128, KT * H)
        )
        tTc = np.ascontiguousarray(
            text[sl].transpose(2, 0, 1)         # [H, BL, Tt]
            .reshape(KT, 128, BL, Tt)
            .transpose(1, 0, 2, 3)              # [128, KT, BL, Tt]
            .reshape(128, KT * BL * Tt)
        )
        hTc = np.ascontiguousarray(
            h[sl].T.reshape(KT, 128, BL).transpose(1, 0, 2).reshape(128, KT * BL)
        )
        f8e = f8e_shared.copy()
        f8e[:, 0:64] = 0.0
        f8e[:, 0:64].reshape(128, KT, 16)[:, :, 0:BL] = hTc.reshape(128, KT, BL)
        f8t_i = f8t.copy()
        f8t_i[:, 0:KT * H] = tTc
        bfs = np.zeros((128, 128 + KT * BL + KT), np.float32)
        bfs[:, 0:128] = np.eye(128, dtype=np.float32)
        bfs[:, 128:128 + KT * BL] = hTc
        bfs[:, 128 + KT * BL:] = np.ascontiguousarray(
            np.asarray(inputs["Vat"], np.float32).reshape(KT, 128).T)
        in_maps.append(dict(
            shared,
            fT8=_f8(fTc), fR8=_f8(fRc),
            f8e=_f8(f8e), f8t=_f8(f8t_i), bfs=_bf(bfs),
        ))
    return in_maps


def run(inputs, trace=False, **kw):
    nc = _get_nc()
    in_maps = make_in_maps(inputs)
    res = run_bass_kernel_spmd(nc, in_maps, core_ids=list(range(NC)), trace=trace, **kw)
    out = np.concatenate([res.results[i]["out"] for i in range(NC)], axis=0)
    return out, res


def kernel(**inputs) -> np.ndarray:
    out, _ = run(inputs, trace=False)
    return out



# revision 18
# speedup vs baseline: 1.0019x; 1.0019x over previous
"""Trainium2 Bass kernel for nn_AttentionTwoStream (two-stream Bahdanau attention
with global gating softmax), distributed over 8 NeuronCores.

Sharding: data-parallel over batch B=64 -> 8 batches per core; weights
replicated. One 16-float-per-core AllGather feeds the global beta softmax
(preceded by a warmup AllGather that hides the ncfw pipeline spin-up).

Compute strategy:
 - frames matmul, score dot and attention einsum run on the TensorEngine in
   fp8e4 DoubleRow mode (2 k-rows per PE pass). Weights that land in fp8 are
   prescaled by 16 (or 256 for gate weights) host-side to clear the fp8
   subnormal floor; descales fold into ACT scale params / the beta scalar.
 - The attention einsum uses exp-unnormalized weights (values ~1.0, fp8-safe)
   against a second [t,h]-layout copy of frames; the softmax divide folds into
   the per-batch evacuation op.
 - gate biases ride the Vector evacuation STTs (no bias matmuls); tanh/exp on
   Scalar; fp32 PSUM accumulation everywhere.
 - DMAs: blobbed by dtype/criticality; sync hwdge queue carries the critical
   score-path set first, scalar/gpsimd queues are gated behind fT[1] so the
   frames stream gets the HBM bandwidth first (per-queue bw is the limit).

Self-contained: hardcodes shapes B=64, Tv=512, Tt=64, H=512, 8 cores.
"""

import numpy as np
import ml_dtypes

import concourse.bacc as bacc
import concourse.mybir as mybir
import concourse.tile as tile
from concourse.bass_utils import run_bass_kernel_spmd

NC = 8          # cores
B = 64          # global batch
BL = B // NC    # batches per core = 8
H = 512
Tv = 512
Tt = 64
KT = H // 128   # 4 contraction tiles (and 4 Tv partition tiles)
F32 = mybir.dt.float32
BF16 = mybir.dt.bfloat16
FP8 = mybir.dt.float8e4
NP_BF16 = ml_dtypes.bfloat16
NP_FP8 = ml_dtypes.float8_e4m3

SW = 16.0       # fp8 prescale for score-path weights
SG = 256.0      # fp8 prescale product for gate/logit-path psums
DR = mybir.MatmulPerfMode.DoubleRow


def build_nc():
    nc = bacc.Bacc(
        "TRN2", target_bir_lowering=False, debug=False,
        enable_asserts=False, num_devices=NC, num_swdge_queues=4,
    )

    def inp(name, shape, dt):
        return nc.dram_tensor(name, list(shape), dt, kind="ExternalInput").ap()

    # --- external inputs (per-core shards; layouts match SBUF tiles) ---
    # fp8 early blob: hT8(64) | vavZ8(64) | uav8 (cols)
    f8e = inp("f8e", (128, 128 + KT * H), FP8)
    wav8 = inp("wav8", (128, KT * H), FP8)
    uat8 = inp("uat8", (128, KT * H), FP8)
    # bf16 smalls blob: eye(128) | hTb(KT*BL) | vatT(KT)
    bfs = inp("bfs", (128, 128 + KT * BL + KT), BF16)
    biasr = inp("biasr", (1, 8 * H), BF16)       # scaled bias rows (warmup fodder)
    biasB = inp("biasB", (BL, 7 * H), BF16)      # per-gate bias rows bcast to 8 parts
    blkI = inp("blkI", (BL, BL * Tt), BF16)      # blkI[b, b'*64+t] = (b==b')
    wbB = inp("wbB", (BL, H), F32)               # wb broadcast to 8 partitions
    fT8 = inp("fT8", (BL, 128, KT * Tv), FP8)    # frames^T [b][p][kt*Tv+t]
    fR8 = inp("fR8", (BL, 128, KT * H), FP8)     # frames   [b][p][tt*H+h], t=tt*128+p
    # fp8 text blob: tT8 | wat8
    f8t = inp("f8t", (128, 2 * KT * H), FP8)
    whh_bf = inp("whh_bf", (128, KT * H), BF16)  # Whh (true scale)
    # fp8 late gate blob: wb8(256x) | vbt8 | wqe8 | vbv8 | wve8 (16x)
    f8g = inp("f8g", (128, 5 * KT * H), FP8)
    out_ext = nc.dram_tensor("out", [BL, H], F32, kind="ExternalOutput").ap()

    ACT = mybir.ActivationFunctionType
    ALU = mybir.AluOpType

    with tile.TileContext(nc) as tc:
        with (
            tc.tile_pool(name="wres", bufs=1) as wres,
            tc.tile_pool(name="work", bufs=4) as work,
            tc.tile_pool(name="small", bufs=1) as small,
            tc.tile_pool(name="psX", bufs=3, space="PSUM") as psX,
            tc.tile_pool(name="psS", bufs=2, space="PSUM") as psS,
            tc.tile_pool(name="psB", bufs=1, space="PSUM") as psB,
            tc.tile_pool(name="psE", bufs=1, space="PSUM") as psE,
            tc.tile_pool(name="psG", bufs=1, space="PSUM") as psG,
            tc.tile_pool(name="dram", bufs=1, space="DRAM") as dram,
        ):
            def load(pool, ap_in, shape, tag, dt, engine, name=None):
                t = pool.tile(list(shape), dt, tag=tag, name=name or tag)
                engine.dma_start(t[:], ap_in)
                return t

            # ---------- warmup collective (primes ncfw; result unused) ----
            warm_out = dram.tile([2 * B, 1], F32, tag="warmout", addr_space="Shared")
            warm_in = dram.tile([2 * BL, 1], F32, tag="warmin")
            nc.gpsimd.collective_compute(
                "AllGather", ALU.bypass,
                replica_groups=[list(range(NC))],
                ins=[warm_in[:].opt()],
                outs=[warm_out[:].opt()],
            )

            # ---------- DMAs ----------
            # pair0 critical set spread across all three queues in parallel:
            # gpsimd pulls fT0/fT1 (4 swdge sub-queues), scalar pulls the
            # P1 fp8 blob, sync pulls wav8 + smalls. Bulk streams follow,
            # gated behind fT1 where they would steal critical bandwidth.
            # sync hwdge:
            biasr_sb = load(wres, biasr, (1, 8 * H), "biasr", BF16, nc.sync)
            wav8_t = load(wres, wav8, (128, KT, H), "wav8", FP8, nc.sync)
            wav8_sb = wav8_t[:, :, :]
            bfs_sb = load(wres, bfs, (128, 128 + KT * BL + KT), "bfs", BF16, nc.sync)
            eye_sb = bfs_sb[:, 0:128]
            hTb_sb = bfs_sb[:, 128:128 + KT * BL].rearrange("p (k b) -> p k b", k=KT)
            vatT_sb = bfs_sb[:, 128 + KT * BL:]
            blkI_sb = load(wres, blkI, (BL, BL * Tt), "blkI", BF16, nc.sync)
            biasB_sb = load(wres, biasB, (BL, 7 * H), "biasB", BF16, nc.sync)
            fT_sb = [None] * BL
            for b in range(2, BL):
                t = wres.tile([128, KT, Tv], FP8, tag=f"fT{b}", name=f"fTs{b}")
                nc.sync.dma_start(t[:], fT8[b])
                fT_sb[b] = t
            wbB_sb = load(wres, wbB, (BL, H), "wbB", F32, nc.sync)
            # gpsimd swdge: f8e + fT0/fT1 first (parallel sub-queues), gated bulk
            f8e_sb = load(wres, f8e, (128, 128 + KT * H), "f8e", FP8, nc.gpsimd)
            hT8_sb = f8e_sb[:, 0:64].rearrange("p (k s) -> p k s", k=KT)
            vavZ8_sb = f8e_sb[:, 64:128].rearrange("p (a b c) -> p a b c", a=2, b=2)
            uav8_sb = f8e_sb[:, 128:].rearrange("p (k n) -> p k n", k=KT)
            for b in range(2):
                t = wres.tile([128, KT, Tv], FP8, tag=f"fT{b}", name=f"fTs{b}")
                nc.gpsimd.dma_start(t[:], fT8[b])
                fT_sb[b] = t
            qgate_g = small.tile([1, 2], FP8, tag="qgate_g")
            nc.gpsimd.tensor_copy(qgate_g[:], fT_sb[1][0:1, 0, 0:2])
            fR_sb = [None] * BL
            for b in range(4):
                t = wres.tile([128, KT, H], FP8, tag=f"fR{b}", name=f"fRs{b}")
                nc.gpsimd.dma_start(t[:], fR8[b])
                fR_sb[b] = t
            f8g_sb = load(wres, f8g, (128, 5 * KT * H), "f8g", FP8, nc.gpsimd)
            wb8_sb = f8g_sb[:, 0:KT * H].rearrange("p (k n) -> p k n", k=KT)
            vbt8_sb = f8g_sb[:, KT * H:2 * KT * H].rearrange("p (k n) -> p k n", k=KT)
            wqe8_sb = f8g_sb[:, 2 * KT * H:3 * KT * H].rearrange("p (k n) -> p k n", k=KT)
            for b in range(4, BL):
                t = wres.tile([128, KT, H], FP8, tag=f"fR{b}", name=f"fRs{b}")
                nc.gpsimd.dma_start(t[:], fR8[b])
                fR_sb[b] = t
            vbv8_sb = f8g_sb[:, 3 * KT * H:4 * KT * H].rearrange("p (k n) -> p k n", k=KT)
            wve8_sb = f8g_sb[:, 4 * KT * H:].rearrange("p (k n) -> p k n", k=KT)
            # scalar hwdge: uat8 first, gated text/late weights
            uat8_t = load(wres, uat8, (128, KT, H), "uat8", FP8, nc.scalar)
            uat8_sb = uat8_t[:, :, :]
            qgate_s = small.tile([1, 2], FP8, tag="qgate_s")
            nc.scalar.copy(qgate_s[:], fT_sb[1][0:1, 0, 0:2])
            f8t_sb = load(wres, f8t, (128, 2 * KT * H), "f8t", FP8, nc.scalar)
            tT8_sb = f8t_sb[:, 0:KT * H].rearrange("p (k n) -> p k n", k=KT)
            wat8_sb = f8t_sb[:, KT * H:].rearrange("p (k n) -> p k n", k=KT)
            whh_sb = load(wres, whh_bf, (128, KT, H), "whh", BF16, nc.scalar)

            ones_sb = small.tile([1, 128], BF16, tag="ones")
            nc.vector.memset(ones_sb[:], 1.0)
            oq_sb = small.tile([1, BL], BF16, tag="oq")
            nc.vector.memset(oq_sb[:], 1.0 / SG)
            onesC_sb = small.tile([128, 1], BF16, tag="onesC")
            nc.vector.memset(onesC_sb[:], 1.0)

            def bB(i):
                return biasB_sb[:, i * H:(i + 1) * H]
            # cols: 16bav, 16bat, 256bve, 256bqe, 256bbv, 256bbt, bh
            bavB, batB, bveB, bqeB, bbvB, bbtB, bhB = (bB(i) for i in range(7))

            # PE warmup: junk matmuls on the first-arriving bytes (p-state
            # ramp while the frames stream loads)
            warm_ps = psX.tile([128, Tv], F32, tag="psX", name="warmps")
            for w in range(6):
                nc.tensor.matmul(
                    warm_ps[0:BL, :], ones_sb[0:1, 0:BL], biasr_sb[0:1, 0:H],
                    start=True, stop=True, skip_group_check=True,
                )

            # fp8 DoubleRow gate: psum[8,H] += (hT|hvT|htT).T @ W  (K=512 via
            # 2 DoubleRow passes), optional extra rows merged by caller.
            def dr_gate(ps, lhs_sb, w_sb, start, stop):
                for ktp in range(2):
                    nc.tensor.matmul(
                        ps[:], lhs_sb[:, 2 * ktp:2 * ktp + 2, 0:BL],
                        w_sb[:, 2 * ktp:2 * ktp + 2, :],
                        start=(start and ktp == 0), stop=(stop and ktp == 1),
                        perf_mode=DR, skip_group_check=True,
                    )

            # ---------- P1: h-projections (score biases) ----------
            uhvb_ps = psG.tile([BL, H], F32, tag="psG", name="uhvb")
            dr_gate(uhvb_ps, hT8_sb, uav8_sb, True, True)
            uhvb_s = small.tile([BL, H], BF16, tag="uhvb_s")
            # uhvb = (16*hUav)/16 + bav
            nc.vector.scalar_tensor_tensor(
                out=uhvb_s[:], in0=uhvb_ps[:], scalar=1.0 / SW, in1=bavB,
                op0=ALU.mult, op1=ALU.add,
            )

            # frames bias in [512,8] layout for per-partition ACT bias
            uhvbT_sb = small.tile([128, KT * BL], F32, tag="uhvbT")
            for jt in range(KT):
                tp = psB.tile([128, BL], BF16, tag="psB", name=f"tpv{jt}")
                nc.tensor.transpose(
                    tp[:], uhvb_s[0:BL, jt * 128:(jt + 1) * 128],
                    eye_sb[0:BL, 0:BL],
                )
                nc.vector.tensor_copy(uhvbT_sb[:, jt * BL:(jt + 1) * BL], tp[:])

            uhtb_ps = psG.tile([BL, H], F32, tag="psG", name="uhtb")
            dr_gate(uhtb_ps, hT8_sb, uat8_sb, True, True)
            uhtb_s = small.tile([BL, H], BF16, tag="uhtb_s")   # 16x scale
            nc.vector.scalar_tensor_tensor(
                out=uhtb_s[:], in0=uhtb_ps[:], scalar=1.0, in1=batB,
                op0=ALU.mult, op1=ALU.add,
            )

            # ---------- frames pair machinery ----------
            NP = BL // 2
            hv16 = small.tile([BL, H], BF16, tag="hv16")
            hv16n = small.tile([BL, H], BF16, tag="hv16n")
            sum8 = small.tile([BL, 1], F32, tag="sum8")
            hvT16 = small.tile([128, KT, 16], FP8, tag="hvT16")
            yv_tiles = {}
            scv_tiles = {}
            avT_tiles = {}
            hv16row = small.tile([1, BL * H], BF16, tag="hv16row")

            def pair_compute(g):
                """xps matmuls + tanh + score accumulation for pair g."""
                scv_g = psS.tile([2, Tv], F32, tag="scS", name=f"scv{g}")
                scv_tiles[g] = scv_g
                nmm = 0
                for jtp in range(2):
                    for i in range(2):
                        b = 2 * g + i
                        yv = work.tile([128, 2, Tv], FP8, tag="yv",
                                       name=f"yv{g}_{jtp}_{i}", bufs=6)
                        for q in range(2):
                            jt = 2 * jtp + q
                            xps = psX.tile([128, Tv], F32, tag="psX",
                                           name=f"xps{g}_{jtp}_{i}_{q}")
                            for ktp in range(2):
                                nc.tensor.matmul(
                                    xps[:],
                                    wav8_sb[:, 2 * ktp:2 * ktp + 2,
                                            jt * 128:(jt + 1) * 128],
                                    fT_sb[b][:, 2 * ktp:2 * ktp + 2, :],
                                    start=(ktp == 0), stop=(ktp == 1),
                                    perf_mode=DR, skip_group_check=True,
                                )
                            # yv = tanh(xps/16 + Uhv + bav)
                            nc.scalar.activation(
                                yv[:, q, :], xps[:], ACT.Tanh,
                                scale=1.0 / SW,
                                bias=uhvbT_sb[:, jt * BL + b: jt * BL + b + 1],
                            )
                        yv_tiles[(g, jtp, i)] = yv
                        nmm += 1
                        nc.tensor.matmul(    # scv += (16Vav).T @ yv -> 16*s
                            scv_g[:],
                            vavZ8_sb[:, jtp, :, 2 * i:2 * i + 2],
                            yv[:, :, :],
                            start=(nmm == 1), stop=(nmm == 4),
                            perf_mode=DR, skip_group_check=True,
                        )

            def pair_chain(g):
                """exp -> transpose weights -> PE einsum -> evac for pair g.
                Normalization is deferred: evacs write 16*unnormalized rows;
                sumv is shipped to sum8 for one post-assembly divide."""
                last = (g == NP - 1)
                tpool, epool = (psX, psX) if last else (psB, psE)
                scv_g = scv_tiles[g]
                avp = small.tile([2, Tv], BF16, tag="avp", name=f"avp{g}", bufs=2)
                sumv = small.tile([2, 1], F32, tag="sumv", name=f"sumv{g}", bufs=2)
                nc.scalar.activation(
                    avp[:], scv_g[:], ACT.Exp, scale=1.0 / SW,
                    accum_out=sumv[:],
                )
                nc.sync.dma_start(sum8[2 * g:2 * g + 2, 0:1], sumv[:, :])
                avT = small.tile([128, KT, 16], FP8, tag="avT", name=f"avT{g}", bufs=2)
                avT_tiles[g] = avT
                for tt in range(KT):
                    tp = tpool.tile([128, 2], BF16, tag=tpool.name, name=f"avtp{g}_{tt}")
                    nc.tensor.transpose(
                        tp[:], avp[0:2, tt * 128:(tt + 1) * 128],
                        eye_sb[0:2, 0:2],
                    )
                    nc.vector.tensor_copy(avT[:, tt, 0:2], tp[:])
                for i in range(2):
                    b = 2 * g + i
                    eps = epool.tile([1, H], F32, tag=epool.name, name=f"eps{g}_{i}")
                    for ttp in range(2):
                        nc.tensor.matmul(
                            eps[:],
                            avT[:, 2 * ttp:2 * ttp + 2, i:i + 1],
                            fR_sb[b][:, 2 * ttp:2 * ttp + 2, :],
                            start=(ttp == 0), stop=(ttp == 1),
                            perf_mode=DR, skip_group_check=True,
                        )
                    # hv16row[b] = 16 * unnormalized weighted sum
                    nc.vector.tensor_scalar_mul(
                        hv16row[0:1, b * H:(b + 1) * H], eps[:], SW,
                    )
                nc.sync.dma_start(
                    hv16[2 * g:2 * g + 2, :],
                    hv16row[0:1, 2 * g * H:(2 * g + 2) * H],
                )

            # --- pair 3 split per-batch: batch 6's chain overlaps batch 7 ---
            scv3 = {}
            avT3 = small.tile([128, KT, 16], FP8, tag="avT", name="avT3", bufs=2)

            def pair3_batch(i):
                b = 6 + i
                scv_b = psS.tile([1, Tv], F32, tag="scS", name=f"scv3_{i}")
                scv3[i] = scv_b
                nmm = 0
                for jtp in range(2):
                    yv = work.tile([128, 2, Tv], FP8, tag="yv",
                                   name=f"yv3_{jtp}_{i}", bufs=6)
                    for q in range(2):
                        jt = 2 * jtp + q
                        xps = psX.tile([128, Tv], F32, tag="psX",
                                       name=f"xps3_{jtp}_{i}_{q}")
                        for ktp in range(2):
                            nc.tensor.matmul(
                                xps[:],
                                wav8_sb[:, 2 * ktp:2 * ktp + 2,
                                        jt * 128:(jt + 1) * 128],
                                fT_sb[b][:, 2 * ktp:2 * ktp + 2, :],
                                start=(ktp == 0), stop=(ktp == 1),
                                perf_mode=DR, skip_group_check=True,
                            )
                        nc.scalar.activation(
                            yv[:, q, :], xps[:], ACT.Tanh,
                            scale=1.0 / SW,
                            bias=uhvbT_sb[:, jt * BL + b: jt * BL + b + 1],
                        )
                    nmm += 1
                    nc.tensor.matmul(   # single-batch score row
                        scv_b[:],
                        vavZ8_sb[:, jtp, :, 0:1],
                        yv[:, :, :],
                        start=(nmm == 1), stop=(nmm == 2),
                        perf_mode=DR, skip_group_check=True,
                    )

            def chain3_i(i):
                # i==0 runs concurrently with batch 7's xps matmuls -> keep it
                # off the psX banks; i==1 runs after all xps -> psX is idle.
                tpool, epool = (psX, psX) if i == 1 else (psB, psE)
                b = 6 + i
                avp = small.tile([1, Tv], BF16, tag="avp", name=f"avp3_{i}", bufs=2)
                sumv = small.tile([1, 1], F32, tag="sumv", name=f"sumv3_{i}", bufs=2)
                nc.scalar.activation(
                    avp[:], scv3[i][:], ACT.Exp, scale=1.0 / SW,
                    accum_out=sumv[:],
                )
                nc.sync.dma_start(sum8[b:b + 1, 0:1], sumv[:, :])
                for tt in range(KT):
                    tp = tpool.tile([128, 1], BF16, tag=tpool.name, name=f"avtp3_{i}_{tt}")
                    nc.tensor.transpose(
                        tp[:], avp[0:1, tt * 128:(tt + 1) * 128],
                        eye_sb[0:1, 0:1],
                    )
                    nc.vector.tensor_copy(avT3[:, tt, i:i + 1], tp[:])
                eps = epool.tile([1, H], F32, tag=epool.name, name=f"eps3_{i}")
                for ttp in range(2):
                    nc.tensor.matmul(
                        eps[:],
                        avT3[:, 2 * ttp:2 * ttp + 2, i:i + 1],
                        fR_sb[b][:, 2 * ttp:2 * ttp + 2, :],
                        start=(ttp == 0), stop=(ttp == 1),
                        perf_mode=DR, skip_group_check=True,
                    )
                nc.vector.tensor_scalar_mul(
                    hv16row[0:1, b * H:(b + 1) * H], eps[:], SW,
                )
                nc.sync.dma_start(hv16[b:b + 1, :], hv16row[0:1, b * H:(b + 1) * H])

            # ---------- text stream pieces ----------
            def text_matmuls():
                sct_ps = psG.tile([1, BL * Tt], F32, tag="psG", name="sct")
                pend = []

                def flush_sct():
                    for yt_, jt_ in pend:
                        nc.tensor.matmul(
                            sct_ps[:], vatT_sb[:, jt_: jt_ + 1], yt_[:],
                            start=(jt_ == 0), stop=(jt_ == KT - 1),
                            skip_group_check=True,
                        )
                    pend.clear()

                for jt in range(KT):
                    xt_ps = psX.tile([128, BL * Tt], F32, tag="psX", name=f"xt{jt}")
                    for ktp in range(2):
                        nc.tensor.matmul(
                            xt_ps[:],
                            wat8_sb[:, 2 * ktp:2 * ktp + 2, jt * 128:(jt + 1) * 128],
                            tT8_sb[:, 2 * ktp:2 * ktp + 2, :],
                            start=(ktp == 0), stop=False,
                            perf_mode=DR, skip_group_check=True,
                        )
                    nc.tensor.matmul(   # bias: += 16*Uhtb[b, jt*128+j] via blkI
                        xt_ps[:], uhtb_s[0:BL, jt * 128:(jt + 1) * 128], blkI_sb[:],
                        start=False, stop=True, skip_group_check=True,
                    )
                    flush_sct()
                    yt = work.tile([128, BL * Tt], BF16, tag="yt", name=f"yt{jt}")
                    nc.scalar.activation(yt[:], xt_ps[:], ACT.Tanh, scale=1.0 / SW)
                    pend.append((yt, jt))
                flush_sct()
                return sct_ps

            def text_softmax(sct_ps):
                sct_sb = small.tile([1, BL * Tt], F32, tag="sct_sb")
                nc.vector.tensor_copy(sct_sb[:], sct_ps[:])
                st8 = small.tile([BL, Tt], F32, tag="st8")
                nc.sync.dma_start(st8[:, :], sct_sb[0:1, :])
                expt = small.tile([BL, Tt], F32, tag="expt")
                sumt = small.tile([BL, 1], F32, tag="sumt")
                nc.scalar.activation(expt[:], st8[:], ACT.Exp, accum_out=sumt[:])
                rt = small.tile([BL, 1], F32, tag="rt")
                nc.vector.reciprocal(rt[:], sumt[:])
                at_sb = small.tile([BL, Tt], BF16, tag="at")
                nc.vector.tensor_scalar_mul(at_sb[:], expt[:], rt[:])
                atRows = small.tile([1, BL * Tt], BF16, tag="atRows")
                nc.sync.dma_start(atRows[0:1, :], at_sb[:, :])
                atB = []
                for b in range(BL):
                    atB_ps = psB.tile([128, Tt], F32, tag="psB", name=f"atB{b}")
                    src = at_sb[0:1, :] if b == 0 else atRows[0:1, b * Tt:(b + 1) * Tt]
                    nc.tensor.matmul(
                        atB_ps[:], ones_sb[0:1, 0:128], src,
                        start=True, stop=True,
                    )
                    t = work.tile([128, Tt], BF16, tag="atB_sb", name=f"atBs{b}", bufs=8)
                    nc.vector.tensor_copy(t[:], atB_ps[:])
                    atB.append(t)
                return atB

            htT_sb = small.tile([128, KT, BL], F32, tag="htT")

            def text_einsum(atB, kts):
                for kt in kts:
                    for b in range(BL):
                        scrt = work.tile([128, Tt], BF16, tag="scrt")
                        nc.vector.scalar_tensor_tensor(
                            out=scrt[:],
                            in0=tT8_sb[:, kt, b * Tt:(b + 1) * Tt],
                            scalar=1.0,
                            in1=atB[b][:],
                            op0=ALU.mult, op1=ALU.mult,
                            accum_out=htT_sb[:, kt, b:b + 1],
                        )

            # ---------- issue order (drives per-engine schedules) ----------
            pair_compute(0)
            sct_ps = text_matmuls()
            pair_compute(1)
            atB = text_softmax(sct_ps)
            pair_chain(0)
            pair_compute(2)
            text_einsum(atB, [0, 1])

            # wbs = 256*(h@Wb); hwhh = h@Whh + bh (true scale)
            wbs_ps = psG.tile([BL, H], F32, tag="psG", name="wbs")
            dr_gate(wbs_ps, hT8_sb, wb8_sb, True, True)
            wbst_sb = small.tile([BL, H], BF16, tag="wbst_sb")  # 256(hWb+bbt)
            nc.vector.scalar_tensor_tensor(
                out=wbst_sb[:], in0=wbs_ps[:], scalar=1.0, in1=bbtB,
                op0=ALU.mult, op1=ALU.add,
            )
            wbsv_sb = small.tile([BL, H], BF16, tag="wbsv_sb")  # 256(hWb+bbv)
            nc.vector.scalar_tensor_tensor(
                out=wbsv_sb[:], in0=wbs_ps[:], scalar=1.0, in1=bbvB,
                op0=ALU.mult, op1=ALU.add,
            )

            pair_chain(1)
            text_einsum(atB, [2, 3])
            pair_chain(2)
            pair3_batch(0)

            hwhh_ps = psG.tile([BL, H], F32, tag="psG", name="hwhh")
            for kt in range(KT):
                nc.tensor.matmul(
                    hwhh_ps[:], hTb_sb[:, kt, :], whh_sb[:, kt, :],
                    start=(kt == 0), stop=(kt == KT - 1), skip_group_check=True,
                )
            hwhh_sb = small.tile([BL, H], F32, tag="hwhh_sb")
            nc.vector.scalar_tensor_tensor(
                out=hwhh_sb[:], in0=hwhh_ps[:], scalar=1.0, in1=bhB,
                op0=ALU.mult, op1=ALU.add,
            )

            chain3_i(0)
            pair3_batch(1)

            # ---------- text gates (256x psums) ----------
            htT16 = small.tile([128, KT, 16], FP8, tag="htT16")
            nc.vector.tensor_scalar_mul(
                htT16[:, :, 0:BL], htT_sb[:, :, :], SW,
            )
            mt1_ps = psG.tile([BL, H], F32, tag="psG", name="mt1")
            nc.tensor.matmul(   # += 256*(h@Wb + bbt) via identity
                mt1_ps[:], eye_sb[0:BL, 0:BL], wbst_sb[:],
                start=True, stop=False, skip_group_check=True,
            )
            dr_gate(mt1_ps, htT16, vbt8_sb, False, True)
            mtv_t = small.tile([BL, H], F32, tag="mtv_t")
            nc.scalar.activation(mtv_t[:], mt1_ps[:], ACT.Tanh, scale=1.0 / SG)
            lgt_t = small.tile([BL, 1], F32, tag="lgt_t")
            scr8t = small.tile([BL, H], F32, tag="scr8t")
            nc.vector.scalar_tensor_tensor(
                out=scr8t[:], in0=mtv_t[:], scalar=1.0, in1=wbB_sb[:],
                op0=ALU.mult, op1=ALU.mult, accum_out=lgt_t[:],
            )
            ht2_ps = psG.tile([BL, H], F32, tag="psG", name="ht2")
            dr_gate(ht2_ps, htT16, wqe8_sb, True, True)
            ht2_sb = small.tile([BL, H], F32, tag="ht2_sb")
            nc.vector.scalar_tensor_tensor(
                out=ht2_sb[:], in0=ht2_ps[:], scalar=1.0, in1=bqeB,
                op0=ALU.mult, op1=ALU.add,
            )

            chain3_i(1)

            # ---------- frames gates + logits ----------
            rv8 = small.tile([BL, 1], F32, tag="rv8")
            nc.vector.reciprocal(rv8[:], sum8[:])
            nc.vector.tensor_scalar_mul(hv16n[:], hv16[:], rv8[:])
            for jt in range(KT):
                tp = psX.tile([128, BL], BF16, tag="psX", name=f"hvtp{jt}")
                nc.tensor.transpose(
                    tp[:], hv16n[0:BL, jt * 128:(jt + 1) * 128],
                    eye_sb[0:BL, 0:BL],
                )
                nc.vector.tensor_copy(hvT16[:, jt, 0:BL], tp[:])

            mv1_ps = psG.tile([BL, H], F32, tag="psG", name="mv1")
            nc.tensor.matmul(
                mv1_ps[:], eye_sb[0:BL, 0:BL], wbsv_sb[:],
                start=True, stop=False, skip_group_check=True,
            )
            dr_gate(mv1_ps, hvT16, vbv8_sb, False, True)
            mtv_v = small.tile([BL, H], F32, tag="mtv_v")
            nc.scalar.activation(mtv_v[:], mv1_ps[:], ACT.Tanh, scale=1.0 / SG)
            lgv_t = small.tile([BL, 1], F32, tag="lgv_t")
            scr8v = small.tile([BL, H], F32, tag="scr8v")
            nc.vector.scalar_tensor_tensor(
                out=scr8v[:], in0=mtv_v[:], scalar=1.0, in1=wbB_sb[:],
                op0=ALU.mult, op1=ALU.mult, accum_out=lgv_t[:],
            )

            # ---------- AllGather of the 16 local logits ----------
            cc_in = dram.tile([2 * BL, 1], F32, tag="ccin")
            cc_out = dram.tile([2 * B, 1], F32, tag="ccout", addr_space="Shared")
            nc.sync.dma_start(cc_in[0:BL], lgv_t[:])
            nc.sync.dma_start(cc_in[BL:2 * BL], lgt_t[:])
            nc.gpsimd.collective_compute(
                "AllGather", ALU.bypass,
                replica_groups=[list(range(NC))],
                ins=[cc_in[:].opt()],
                outs=[cc_out[:].opt()],
            )

            # overlap the AG: hv2 = 256*(hv@Wve.T + bve)
            hv2_ps = psG.tile([BL, H], F32, tag="psG", name="hv2")
            dr_gate(hv2_ps, hvT16, wve8_sb, True, True)
            hv2_sb = small.tile([BL, H], F32, tag="hv2_sb")
            nc.vector.scalar_tensor_tensor(
                out=hv2_sb[:], in0=hv2_ps[:], scalar=1.0, in1=bveB,
                op0=ALU.mult, op1=ALU.add,
            )

            # ---------- global beta softmax (logits tiny; no max-shift) ----
            g_col = small.tile([2 * B, 1], F32, tag="g")
            nc.sync.dma_start(g_col[:, :], cc_out[:, :])
            ge_col = small.tile([2 * B, 1], BF16, tag="ge")
            nc.scalar.activation(ge_col[:], g_col[:], ACT.Exp)
            z_ps = psE.tile([1, 1], F32, tag="psE", name="zps")
            nc.tensor.matmul(   # Z = sum over partitions
                z_ps[:], onesC_sb[0:128, 0:1], ge_col[0:128, :],
                start=True, stop=True, skip_group_check=True,
            )
            geR_ps = psB.tile([1, 2], BF16, tag="psB", name="geR")
            nc.tensor.transpose(geR_ps[:], ge_col[0:2, 0:1], eye_sb[0:2, 0:2])
            rg = small.tile([1, 1], F32, tag="rg")
            nc.vector.reciprocal(rg[:], z_ps[:])
            betas = small.tile([1, 2], BF16, tag="betas")
            nc.vector.tensor_scalar_mul(betas[:], geR_ps[:], rg[:])
            beta8_ps = psB.tile([BL, 2], F32, tag="psB", name="beta8")
            nc.tensor.matmul(   # beta/256 broadcast to 8 partitions
                beta8_ps[:], oq_sb[0:1, 0:BL], betas[0:1, 0:2],
                start=True, stop=True,
            )
            # ---------- out = tanh(hWhh+bh + b0*hv2 + b1*ht2) ----------
            t1 = small.tile([BL, H], F32, tag="t1")
            nc.vector.scalar_tensor_tensor(
                out=t1[:], in0=hv2_sb[:], scalar=beta8_ps[:, 0:1], in1=hwhh_sb[:],
                op0=ALU.mult, op1=ALU.add,
            )
            s1 = small.tile([BL, H], F32, tag="s1")
            nc.vector.scalar_tensor_tensor(
                out=s1[:], in0=ht2_sb[:], scalar=beta8_ps[:, 1:2], in1=t1[:],
                op0=ALU.mult, op1=ALU.add,
            )
            out_sb = small.tile([BL, H], F32, tag="out_sb")
            nc.scalar.activation(out_sb[:], s1[:], ACT.Tanh)
            nc.sync.dma_start(out_ext, out_sb[:])

    nc.compile()
    return nc


_cached_nc = None


def _get_nc():
    global _cached_nc
    if _cached_nc is None:
        _cached_nc = build_nc()
    return _cached_nc


def _bf(a):
    return np.asarray(a, np.float32).astype(NP_BF16)


def _f8(a):
    return np.asarray(a, np.float32).astype(NP_FP8)


def _pack_w(w):
    """[512,512] -> [128, 4*512] with free = kt*512 + j, k = kt*128 + p."""
    return np.ascontiguousarray(
        np.asarray(w, np.float32).reshape(KT, 128, H).transpose(1, 0, 2)
        .reshape(128, KT * H)
    )


def make_in_maps(inputs):
    h = np.asarray(inputs["h"], np.float32)
    frames = np.asarray(inputs["hidden_frames"], np.float32)
    text = np.asarray(inputs["hidden_text"], np.float32)

    Vav = np.asarray(inputs["Vav"], np.float32)
    Vat = np.asarray(inputs["Vat"], np.float32)
    wb = np.asarray(inputs["wb"], np.float32)

    # vavZ8[p, jtp, q, 2v+c] = 16*Vav[(2jtp+q)*128+p] if c == v (pad to 16)
    vavZ = np.zeros((128, 2, 2, 16), np.float32)
    for jtp in range(2):
        for q in range(2):
            col = Vav[(2 * jtp + q) * 128:(2 * jtp + q + 1) * 128] * SW
            for v in range(2):
                vavZ[:, jtp, q, 2 * v + v] = col

    biasr = np.zeros((1, 8 * H), np.float32)
    biasr = _bf(biasr)
    biasB = np.zeros((BL, 7 * H), np.float32)
    scales = [SW, SW, SG, SG, SG, SG, 1.0]
    for i, (k, s) in enumerate(zip(
            ["bav", "bat", "bve", "bqe", "bbv", "bbt", "bh"], scales)):
        biasB[:, i * H:(i + 1) * H] = np.asarray(inputs[k], np.float32)[None, :] * s
    biasB = _bf(biasB)
    wbB = np.ascontiguousarray(np.broadcast_to(wb, (BL, H))).astype(np.float32)
    blkI = np.zeros((BL, BL, Tt), np.float32)
    for b in range(BL):
        blkI[b, b, :] = 1.0
    blkI = _bf(blkI.reshape(BL, BL * Tt))

    hTp = np.zeros((128, KT, 16), np.float32)   # per-core filled below
    f8e_shared = np.zeros((128, 128 + KT * H), np.float32)
    f8e_shared[:, 64:128] = vavZ.reshape(128, 64)
    f8e_shared[:, 128:] = _pack_w(inputs["Uav"]) * SW
    wav8 = _f8(_pack_w(inputs["Wav"]) * SW)
    uat8 = _f8(_pack_w(inputs["Uat"]) * SW)

    f8t = np.zeros((128, 2 * KT * H), np.float32)
    # tT8 filled per-core below
    f8t[:, KT * H:] = _pack_w(inputs["Wat"]) * SW

    f8g = np.zeros((128, 5 * KT * H), np.float32)
    f8g[:, 0:KT * H] = _pack_w(inputs["Wb"]) * SG
    f8g[:, KT * H:2 * KT * H] = _pack_w(inputs["Vbt"]) * SW
    f8g[:, 2 * KT * H:3 * KT * H] = _pack_w(np.asarray(inputs["Wqe"], np.float32).T) * SW
    f8g[:, 3 * KT * H:4 * KT * H] = _pack_w(inputs["Vbv"]) * SW
    f8g[:, 4 * KT * H:] = _pack_w(np.asarray(inputs["Wve"], np.float32).T) * SW
    f8g = _f8(f8g)

    shared = dict(
        f8g=f8g, wav8=wav8, uat8=uat8,
        whh_bf=_bf(_pack_w(inputs["Whh"])),
        biasr=biasr, biasB=biasB, wbB=wbB, blkI=blkI,
    )

    in_maps = []
    for i in range(NC):
        sl = slice(i * BL, (i + 1) * BL)
        fTc = np.ascontiguousarray(
            frames[sl].transpose(0, 2, 1)       # [BL, H, Tv]
            .reshape(BL, KT, 128, Tv)
            .transpose(0, 2, 1, 3)              # [BL, 128, KT, Tv]
            .reshape(BL, 128, KT * Tv)
        )
        fRc = np.ascontiguousarray(
            frames[sl]                          # [BL, Tv, H]
            .reshape(BL, KT, 128, H)
            .transpose(0, 2, 1, 3)              # [BL, 128, TT, H]
            .reshape(BL, 128, KT * H)
        )
        tTc = np.ascontiguousarray(
            text[sl].transpose(2, 0, 1)         # [H, BL, Tt]
            .reshape(KT, 128, BL, Tt)
            .transpose(1, 0, 2, 3)              # [128, KT, BL, Tt]
            .reshape(128, KT * BL * Tt)
        )
        hTc = np.ascontiguousarray(
            h[sl].T.reshape(KT, 128, BL).transpose(1, 0, 2).reshape(128, KT * BL)
        )
        f8e = f8e_shared.copy()
        f8e[:, 0:64] = 0.0
        f8e[:, 0:64].reshape(128, KT, 16)[:, :, 0:BL] = hTc.reshape(128, KT, BL)
        f8t_i = f8t.copy()
        f8t_i[:, 0:KT * H] = tTc
        bfs = np.zeros((128, 128 + KT * BL + KT), np.float32)
        bfs[:, 0:128] = np.eye(128, dtype=np.float32)
        bfs[:, 128:128 + KT * BL] = hTc
        bfs[:, 128 + KT * BL:] = np.ascontiguousarray(
            np.asarray(inputs["Vat"], np.float32).reshape(KT, 128).T)
        in_maps.append(dict(
            shared,
            fT8=_f8(fTc), fR8=_f8(fRc),
            f8e=_f8(f8e), f8t=_f8(f8t_i), bfs=_bf(bfs),
        ))
    return in_maps


def run(inputs, trace=False, **kw):
    nc = _get_nc()
    in_maps = make_in_maps(inputs)
    res = run_bass_kernel_spmd(nc, in_maps, core_ids=list(range(NC)), trace=trace, **kw)
    out = np.concatenate([res.results[i]["out"] for i in range(NC)], axis=0)
    return out, res


def kernel(**inputs) -> np.ndarray:
    out, _ = run(inputs, trace=False)
    return out



# revision 20
# speedup vs baseline: 1.0952x; 1.0932x over previous
"""Trainium2 Bass kernel for nn_AttentionTwoStream (two-stream Bahdanau attention
with global gating softmax), distributed over 8 NeuronCores.

Sharding: data-parallel over batch B=64 -> 8 batches per core; weights
replicated. One 16-float-per-core AllGather feeds the global beta softmax
(preceded by a warmup AllGather that hides the ncfw pipeline spin-up).

Compute strategy:
 - frames matmul, score dot and attention einsum run on the TensorEngine in
   fp8e4 DoubleRow mode (2 k-rows per PE pass). Weights that land in fp8 are
   prescaled by 16 (or 256 for gate weights) host-side to clear the fp8
   subnormal floor; descales fold into ACT scale params / the beta scalar.
 - The attention einsum uses exp-unnormalized weights (values ~1.0, fp8-safe)
   against a second [t,h]-layout copy of frames; the softmax divide folds into
   the per-batch evacuation op.
 - gate biases ride the Vector evacuation STTs (no bias matmuls); tanh/exp on
   Scalar; fp32 PSUM accumulation everywhere.
 - DMAs: blobbed by dtype/criticality; sync hwdge queue carries the critical
   score-path set first, scalar/gpsimd queues are gated behind fT[1] so the
   frames stream gets the HBM bandwidth first (per-queue bw is the limit).

Self-contained: hardcodes shapes B=64, Tv=512, Tt=64, H=512, 8 cores.
"""

import numpy as np
import ml_dtypes

import concourse.bacc as bacc
import concourse.mybir as mybir
import concourse.tile as tile
from concourse.bass_utils import run_bass_kernel_spmd

NC = 8          # cores
B = 64          # global batch
BL = B // NC    # batches per core = 8
H = 512
Tv = 512
Tt = 64
KT = H // 128   # 4 contraction tiles (and 4 Tv partition tiles)
F32 = mybir.dt.float32
BF16 = mybir.dt.bfloat16
FP8 = mybir.dt.float8e4
NP_BF16 = ml_dtypes.bfloat16
NP_FP8 = ml_dtypes.float8_e4m3

SW = 16.0       # fp8 prescale for score-path weights
SG = 256.0      # fp8 prescale product for gate/logit-path psums
DR = mybir.MatmulPerfMode.DoubleRow


def build_nc():
    nc = bacc.Bacc(
        "TRN2", target_bir_lowering=False, debug=False,
        enable_asserts=False, num_devices=NC, num_swdge_queues=4,
    )

    def inp(name, shape, dt):
        return nc.dram_tensor(name, list(shape), dt, kind="ExternalInput").ap()

    # --- external inputs (per-core shards; layouts match SBUF tiles) ---
    # fp8 early blob: hT8(64) | vavZ8(64) | uav8 (cols)
    f8e = inp("f8e", (128, 128 + KT * H), FP8)
    wav8 = inp("wav8", (128, KT * H), FP8)
    uat8 = inp("uat8", (128, KT * H), FP8)
    # bf16 smalls blob: eye(128) | hTb(KT*BL) | vatT(KT)
    bfs = inp("bfs", (128, 128 + KT * BL + KT), BF16)
    biasr = inp("biasr", (1, 8 * H), BF16)       # scaled bias rows (warmup fodder)
    biasB = inp("biasB", (BL, 7 * H), BF16)      # per-gate bias rows bcast to 8 parts
    blkI = inp("blkI", (BL, BL * Tt), BF16)      # blkI[b, b'*64+t] = (b==b')
    wbB = inp("wbB", (BL, H), F32)               # wb broadcast to 8 partitions
    fT8 = inp("fT8", (BL, 128, KT * Tv), FP8)    # frames^T [b][p][kt*Tv+t]
    fR8 = inp("fR8", (BL, 128, KT * H), FP8)     # frames   [b][p][tt*H+h], t=tt*128+p
    # fp8 text blob: tT8 | wat8
    f8t = inp("f8t", (128, 2 * KT * H), FP8)
    whh_bf = inp("whh_bf", (128, KT * H), BF16)  # Whh (true scale)
    # fp8 late gate blob: wb8(256x) | vbt8 | wqe8 | vbv8 | wve8 (16x)
    f8g = inp("f8g", (128, 5 * KT * H), FP8)
    sel3 = inp("sel3", (BL, 6), BF16)     # payload selectors (core-0 mask)
    sel32 = inp("sel32", (4 * NC, 3), F32)  # post-AG row selector
    out_ext = nc.dram_tensor("out", [BL, H], F32, kind="ExternalOutput").ap()

    ACT = mybir.ActivationFunctionType
    ALU = mybir.AluOpType

    with tile.TileContext(nc) as tc:
        with (
            tc.tile_pool(name="wres", bufs=1) as wres,
            tc.tile_pool(name="work", bufs=4) as work,
            tc.tile_pool(name="small", bufs=1) as small,
            tc.tile_pool(name="psX", bufs=3, space="PSUM") as psX,
            tc.tile_pool(name="psS", bufs=2, space="PSUM") as psS,
            tc.tile_pool(name="psB", bufs=1, space="PSUM") as psB,
            tc.tile_pool(name="psE", bufs=1, space="PSUM") as psE,
            tc.tile_pool(name="psG", bufs=1, space="PSUM") as psG,
            tc.tile_pool(name="dram", bufs=1, space="DRAM") as dram,
        ):
            def load(pool, ap_in, shape, tag, dt, engine, name=None):
                t = pool.tile(list(shape), dt, tag=tag, name=name or tag)
                engine.dma_start(t[:], ap_in)
                return t

            # ---------- warmup collective (primes ncfw; result unused) ----
            warm_out = dram.tile([2 * B, 1], F32, tag="warmout", addr_space="Shared")
            warm_in = dram.tile([2 * BL, 1], F32, tag="warmin")
            nc.gpsimd.collective_compute(
                "AllGather", ALU.bypass,
                replica_groups=[list(range(NC))],
                ins=[warm_in[:].opt()],
                outs=[warm_out[:].opt()],
            )

            # ---------- DMAs ----------
            # pair0 critical set spread across all three queues in parallel:
            # gpsimd pulls fT0/fT1 (4 swdge sub-queues), scalar pulls the
            # P1 fp8 blob, sync pulls wav8 + smalls. Bulk streams follow,
            # gated behind fT1 where they would steal critical bandwidth.
            # sync hwdge:
            biasr_sb = load(wres, biasr, (1, 8 * H), "biasr", BF16, nc.sync)
            wav8_t = load(wres, wav8, (128, KT, H), "wav8", FP8, nc.sync)
            wav8_sb = wav8_t[:, :, :]
            bfs_sb = load(wres, bfs, (128, 128 + KT * BL + KT), "bfs", BF16, nc.sync)
            eye_sb = bfs_sb[:, 0:128]
            hTb_sb = bfs_sb[:, 128:128 + KT * BL].rearrange("p (k b) -> p k b", k=KT)
            vatT_sb = bfs_sb[:, 128 + KT * BL:]
            blkI_sb = load(wres, blkI, (BL, BL * Tt), "blkI", BF16, nc.sync)
            biasB_sb = load(wres, biasB, (BL, 7 * H), "biasB", BF16, nc.sync)
            fT_sb = [None] * BL
            for b in range(2, BL):
                t = wres.tile([128, KT, Tv], FP8, tag=f"fT{b}", name=f"fTs{b}")
                nc.sync.dma_start(t[:], fT8[b])
                fT_sb[b] = t
            wbB_sb = load(wres, wbB, (BL, H), "wbB", F32, nc.sync)
            sel3_sb = load(small, sel3, (BL, 6), "sel3", BF16, nc.sync)
            sel32_sb = load(small, sel32, (4 * NC, 3), "sel32", F32, nc.sync)
            # gpsimd swdge: f8e + fT0/fT1 first (parallel sub-queues), gated bulk
            f8e_sb = load(wres, f8e, (128, 128 + KT * H), "f8e", FP8, nc.gpsimd)
            hT8_sb = f8e_sb[:, 0:64].rearrange("p (k s) -> p k s", k=KT)
            vavZ8_sb = f8e_sb[:, 64:128].rearrange("p (a b c) -> p a b c", a=2, b=2)
            uav8_sb = f8e_sb[:, 128:].rearrange("p (k n) -> p k n", k=KT)
            for b in range(2):
                t = wres.tile([128, KT, Tv], FP8, tag=f"fT{b}", name=f"fTs{b}")
                nc.gpsimd.dma_start(t[:], fT8[b])
                fT_sb[b] = t
            qgate_g = small.tile([1, 2], FP8, tag="qgate_g")
            nc.gpsimd.tensor_copy(qgate_g[:], fT_sb[1][0:1, 0, 0:2])
            fR_sb = [None] * BL
            for b in range(4):
                t = wres.tile([128, KT, H], FP8, tag=f"fR{b}", name=f"fRs{b}")
                nc.gpsimd.dma_start(t[:], fR8[b])
                fR_sb[b] = t
            f8g_sb = load(wres, f8g, (128, 5 * KT * H), "f8g", FP8, nc.gpsimd)
            wb8_sb = f8g_sb[:, 0:KT * H].rearrange("p (k n) -> p k n", k=KT)
            vbt8_sb = f8g_sb[:, KT * H:2 * KT * H].rearrange("p (k n) -> p k n", k=KT)
            wqe8_sb = f8g_sb[:, 2 * KT * H:3 * KT * H].rearrange("p (k n) -> p k n", k=KT)
            for b in range(4, BL):
                t = wres.tile([128, KT, H], FP8, tag=f"fR{b}", name=f"fRs{b}")
                nc.gpsimd.dma_start(t[:], fR8[b])
                fR_sb[b] = t
            vbv8_sb = f8g_sb[:, 3 * KT * H:4 * KT * H].rearrange("p (k n) -> p k n", k=KT)
            wve8_sb = f8g_sb[:, 4 * KT * H:].rearrange("p (k n) -> p k n", k=KT)
            # scalar hwdge: uat8 first, gated text/late weights
            uat8_t = load(wres, uat8, (128, KT, H), "uat8", FP8, nc.scalar)
            uat8_sb = uat8_t[:, :, :]
            qgate_s = small.tile([1, 2], FP8, tag="qgate_s")
            nc.scalar.copy(qgate_s[:], fT_sb[1][0:1, 0, 0:2])
            f8t_sb = load(wres, f8t, (128, 2 * KT * H), "f8t", FP8, nc.scalar)
            tT8_sb = f8t_sb[:, 0:KT * H].rearrange("p (k n) -> p k n", k=KT)
            wat8_sb = f8t_sb[:, KT * H:].rearrange("p (k n) -> p k n", k=KT)
            whh_sb = load(wres, whh_bf, (128, KT, H), "whh", BF16, nc.scalar)

            ones_sb = small.tile([1, 128], BF16, tag="ones")
            nc.vector.memset(ones_sb[:], 1.0)
            oq_sb = small.tile([1, BL], BF16, tag="oq")
            nc.vector.memset(oq_sb[:], 1.0 / SG)
            onesC_sb = small.tile([128, 1], BF16, tag="onesC")
            nc.vector.memset(onesC_sb[:], 1.0)

            def bB(i):
                return biasB_sb[:, i * H:(i + 1) * H]
            # cols: 16bav, 16bat, 256bve, 256bqe, 256bbv, 256bbt, bh
            bavB, batB, bveB, bqeB, bbvB, bbtB, bhB = (bB(i) for i in range(7))

            # PE warmup: junk matmuls on the first-arriving bytes (p-state
            # ramp while the frames stream loads)
            warm_ps = psX.tile([128, Tv], F32, tag="psX", name="warmps")
            for w in range(6):
                nc.tensor.matmul(
                    warm_ps[0:BL, :], ones_sb[0:1, 0:BL], biasr_sb[0:1, 0:H],
                    start=True, stop=True, skip_group_check=True,
                )

            # fp8 DoubleRow gate: psum[8,H] += (hT|hvT|htT).T @ W  (K=512 via
            # 2 DoubleRow passes), optional extra rows merged by caller.
            def dr_gate(ps, lhs_sb, w_sb, start, stop):
                for ktp in range(2):
                    nc.tensor.matmul(
                        ps[:], lhs_sb[:, 2 * ktp:2 * ktp + 2, 0:BL],
                        w_sb[:, 2 * ktp:2 * ktp + 2, :],
                        start=(start and ktp == 0), stop=(stop and ktp == 1),
                        perf_mode=DR, skip_group_check=True,
                    )

            # ---------- P1: h-projections (score biases) ----------
            uhvb_ps = psG.tile([BL, H], F32, tag="psG", name="uhvb")
            dr_gate(uhvb_ps, hT8_sb, uav8_sb, True, True)
            uhvb_s = small.tile([BL, H], BF16, tag="uhvb_s")
            # uhvb = (16*hUav)/16 + bav
            nc.vector.scalar_tensor_tensor(
                out=uhvb_s[:], in0=uhvb_ps[:], scalar=1.0 / SW, in1=bavB,
                op0=ALU.mult, op1=ALU.add,
            )

            # frames bias in [512,8] layout for per-partition ACT bias
            uhvbT_sb = small.tile([128, KT * BL], F32, tag="uhvbT")
            for jt in range(KT):
                tp = psB.tile([128, BL], BF16, tag="psB", name=f"tpv{jt}")
                nc.tensor.transpose(
                    tp[:], uhvb_s[0:BL, jt * 128:(jt + 1) * 128],
                    eye_sb[0:BL, 0:BL],
                )
                nc.vector.tensor_copy(uhvbT_sb[:, jt * BL:(jt + 1) * BL], tp[:])

            uhtb_ps = psG.tile([BL, H], F32, tag="psG", name="uhtb")
            dr_gate(uhtb_ps, hT8_sb, uat8_sb, True, True)
            uhtb_s = small.tile([BL, H], BF16, tag="uhtb_s")   # 16x scale
            nc.vector.scalar_tensor_tensor(
                out=uhtb_s[:], in0=uhtb_ps[:], scalar=1.0, in1=batB,
                op0=ALU.mult, op1=ALU.add,
            )

            # ---------- frames pair machinery ----------
            NP = BL // 2
            hv16 = small.tile([BL, H], BF16, tag="hv16")
            hv16n = small.tile([BL, H], BF16, tag="hv16n")
            sum8 = small.tile([BL, 1], F32, tag="sum8")
            hvT16 = small.tile([128, KT, 16], FP8, tag="hvT16")
            yv_tiles = {}
            scv_tiles = {}
            avT_tiles = {}
            hv16row = small.tile([1, BL * H], BF16, tag="hv16row")

            def pair_compute(g):
                """xps matmuls + tanh + score accumulation for pair g."""
                scv_g = psS.tile([2, Tv], F32, tag="scS", name=f"scv{g}")
                scv_tiles[g] = scv_g
                nmm = 0
                for jtp in range(2):
                    for i in range(2):
                        b = 2 * g + i
                        yv = work.tile([128, 2, Tv], FP8, tag="yv",
                                       name=f"yv{g}_{jtp}_{i}", bufs=6)
                        for q in range(2):
                            jt = 2 * jtp + q
                            xps = psX.tile([128, Tv], F32, tag="psX",
                                           name=f"xps{g}_{jtp}_{i}_{q}")
                            for ktp in range(2):
                                nc.tensor.matmul(
                                    xps[:],
                                    wav8_sb[:, 2 * ktp:2 * ktp + 2,
                                            jt * 128:(jt + 1) * 128],
                                    fT_sb[b][:, 2 * ktp:2 * ktp + 2, :],
                                    start=(ktp == 0), stop=(ktp == 1),
                                    perf_mode=DR, skip_group_check=True,
                                )
                            # yv = tanh(xps/16 + Uhv + bav)
                            nc.scalar.activation(
                                yv[:, q, :], xps[:], ACT.Tanh,
                                scale=1.0 / SW,
                                bias=uhvbT_sb[:, jt * BL + b: jt * BL + b + 1],
                            )
                        yv_tiles[(g, jtp, i)] = yv
                        nmm += 1
                        nc.tensor.matmul(    # scv += (16Vav).T @ yv -> 16*s
                            scv_g[:],
                            vavZ8_sb[:, jtp, :, 2 * i:2 * i + 2],
                            yv[:, :, :],
                            start=(nmm == 1), stop=(nmm == 4),
                            perf_mode=DR, skip_group_check=True,
                        )

            def pair_chain(g):
                """exp -> transpose weights -> PE einsum -> evac for pair g.
                Normalization is deferred: evacs write 16*unnormalized rows;
                sumv is shipped to sum8 for one post-assembly divide."""
                last = (g == NP - 1)
                tpool, epool = (psX, psX) if last else (psB, psE)
                scv_g = scv_tiles[g]
                avp = small.tile([2, Tv], BF16, tag="avp", name=f"avp{g}", bufs=2)
                sumv = small.tile([2, 1], F32, tag="sumv", name=f"sumv{g}", bufs=2)
                nc.scalar.activation(
                    avp[:], scv_g[:], ACT.Exp, scale=1.0 / SW,
                    accum_out=sumv[:],
                )
                nc.sync.dma_start(sum8[2 * g:2 * g + 2, 0:1], sumv[:, :])
                avT = small.tile([128, KT, 16], FP8, tag="avT", name=f"avT{g}", bufs=2)
                avT_tiles[g] = avT
                for tt in range(KT):
                    tp = tpool.tile([128, 2], BF16, tag=tpool.name, name=f"avtp{g}_{tt}")
                    nc.tensor.transpose(
                        tp[:], avp[0:2, tt * 128:(tt + 1) * 128],
                        eye_sb[0:2, 0:2],
                    )
                    nc.vector.tensor_copy(avT[:, tt, 0:2], tp[:])
                for i in range(2):
                    b = 2 * g + i
                    eps = epool.tile([1, H], F32, tag=epool.name, name=f"eps{g}_{i}")
                    for ttp in range(2):
                        nc.tensor.matmul(
                            eps[:],
                            avT[:, 2 * ttp:2 * ttp + 2, i:i + 1],
                            fR_sb[b][:, 2 * ttp:2 * ttp + 2, :],
                            start=(ttp == 0), stop=(ttp == 1),
                            perf_mode=DR, skip_group_check=True,
                        )
                    # hv16row[b] = 16 * unnormalized weighted sum
                    nc.vector.tensor_scalar_mul(
                        hv16row[0:1, b * H:(b + 1) * H], eps[:], SW,
                    )
                nc.sync.dma_start(
                    hv16[2 * g:2 * g + 2, :],
                    hv16row[0:1, 2 * g * H:(2 * g + 2) * H],
                )

            # --- pair 3 split per-batch: batch 6's chain overlaps batch 7 ---
            scv3 = {}
            avT3 = small.tile([128, KT, 16], FP8, tag="avT", name="avT3", bufs=2)

            def pair3_batch(i):
                b = 6 + i
                scv_b = psS.tile([1, Tv], F32, tag="scS", name=f"scv3_{i}")
                scv3[i] = scv_b
                nmm = 0
                for jtp in range(2):
                    yv = work.tile([128, 2, Tv], FP8, tag="yv",
                                   name=f"yv3_{jtp}_{i}", bufs=6)
                    for q in range(2):
                        jt = 2 * jtp + q
                        xps = psX.tile([128, Tv], F32, tag="psX",
                                       name=f"xps3_{jtp}_{i}_{q}")
                        for ktp in range(2):
                            nc.tensor.matmul(
                                xps[:],
                                wav8_sb[:, 2 * ktp:2 * ktp + 2,
                                        jt * 128:(jt + 1) * 128],
                                fT_sb[b][:, 2 * ktp:2 * ktp + 2, :],
                                start=(ktp == 0), stop=(ktp == 1),
                                perf_mode=DR, skip_group_check=True,
                            )
                        nc.scalar.activation(
                            yv[:, q, :], xps[:], ACT.Tanh,
                            scale=1.0 / SW,
                            bias=uhvbT_sb[:, jt * BL + b: jt * BL + b + 1],
                        )
                    nmm += 1
                    nc.tensor.matmul(   # single-batch score row
                        scv_b[:],
                        vavZ8_sb[:, jtp, :, 0:1],
                        yv[:, :, :],
                        start=(nmm == 1), stop=(nmm == 2),
                        perf_mode=DR, skip_group_check=True,
                    )

            def chain3_i(i):
                # i==0 runs concurrently with batch 7's xps matmuls -> keep it
                # off the psX banks; i==1 runs after all xps -> psX is idle.
                tpool, epool = (psX, psX) if i == 1 else (psB, psE)
                b = 6 + i
                avp = small.tile([1, Tv], BF16, tag="avp", name=f"avp3_{i}", bufs=2)
                sumv = small.tile([1, 1], F32, tag="sumv", name=f"sumv3_{i}", bufs=2)
                nc.scalar.activation(
                    avp[:], scv3[i][:], ACT.Exp, scale=1.0 / SW,
                    accum_out=sumv[:],
                )
                nc.sync.dma_start(sum8[b:b + 1, 0:1], sumv[:, :])
                for tt in range(KT):
                    tp = tpool.tile([128, 1], BF16, tag=tpool.name, name=f"avtp3_{i}_{tt}")
                    nc.tensor.transpose(
                        tp[:], avp[0:1, tt * 128:(tt + 1) * 128],
                        eye_sb[0:1, 0:1],
                    )
                    nc.vector.tensor_copy(avT3[:, tt, i:i + 1], tp[:])
                eps = epool.tile([1, H], F32, tag=epool.name, name=f"eps3_{i}")
                for ttp in range(2):
                    nc.tensor.matmul(
                        eps[:],
                        avT3[:, 2 * ttp:2 * ttp + 2, i:i + 1],
                        fR_sb[b][:, 2 * ttp:2 * ttp + 2, :],
                        start=(ttp == 0), stop=(ttp == 1),
                        perf_mode=DR, skip_group_check=True,
                    )
                nc.vector.tensor_scalar_mul(
                    hv16row[0:1, b * H:(b + 1) * H], eps[:], SW,
                )
                nc.sync.dma_start(hv16[b:b + 1, :], hv16row[0:1, b * H:(b + 1) * H])

            # ---------- text stream pieces ----------
            def text_matmuls():
                sct_ps = psG.tile([1, BL * Tt], F32, tag="psG", name="sct")
                pend = []

                def flush_sct():
                    for yt_, jt_ in pend:
                        nc.tensor.matmul(
                            sct_ps[:], vatT_sb[:, jt_: jt_ + 1], yt_[:],
                            start=(jt_ == 0), stop=(jt_ == KT - 1),
                            skip_group_check=True,
                        )
                    pend.clear()

                for jt in range(KT):
                    xt_ps = psX.tile([128, BL * Tt], F32, tag="psX", name=f"xt{jt}")
                    for ktp in range(2):
                        nc.tensor.matmul(
                            xt_ps[:],
                            wat8_sb[:, 2 * ktp:2 * ktp + 2, jt * 128:(jt + 1) * 128],
                            tT8_sb[:, 2 * ktp:2 * ktp + 2, :],
                            start=(ktp == 0), stop=False,
                            perf_mode=DR, skip_group_check=True,
                        )
                    nc.tensor.matmul(   # bias: += 16*Uhtb[b, jt*128+j] via blkI
                        xt_ps[:], uhtb_s[0:BL, jt * 128:(jt + 1) * 128], blkI_sb[:],
                        start=False, stop=True, skip_group_check=True,
                    )
                    flush_sct()
                    yt = work.tile([128, BL * Tt], BF16, tag="yt", name=f"yt{jt}")
                    nc.scalar.activation(yt[:], xt_ps[:], ACT.Tanh, scale=1.0 / SW)
                    pend.append((yt, jt))
                flush_sct()
                return sct_ps

            def text_softmax(sct_ps):
                sct_sb = small.tile([1, BL * Tt], F32, tag="sct_sb")
                nc.vector.tensor_copy(sct_sb[:], sct_ps[:])
                st8 = small.tile([BL, Tt], F32, tag="st8")
                nc.sync.dma_start(st8[:, :], sct_sb[0:1, :])
                expt = small.tile([BL, Tt], F32, tag="expt")
                sumt = small.tile([BL, 1], F32, tag="sumt")
                nc.scalar.activation(expt[:], st8[:], ACT.Exp, accum_out=sumt[:])
                rt = small.tile([BL, 1], F32, tag="rt")
                nc.vector.reciprocal(rt[:], sumt[:])
                at_sb = small.tile([BL, Tt], BF16, tag="at")
                nc.vector.tensor_scalar_mul(at_sb[:], expt[:], rt[:])
                atRows = small.tile([1, BL * Tt], BF16, tag="atRows")
                nc.sync.dma_start(atRows[0:1, :], at_sb[:, :])
                atB = []
                for b in range(BL):
                    atB_ps = psB.tile([128, Tt], F32, tag="psB", name=f"atB{b}")
                    src = at_sb[0:1, :] if b == 0 else atRows[0:1, b * Tt:(b + 1) * Tt]
                    nc.tensor.matmul(
                        atB_ps[:], ones_sb[0:1, 0:128], src,
                        start=True, stop=True,
                    )
                    t = work.tile([128, Tt], BF16, tag="atB_sb", name=f"atBs{b}", bufs=8)
                    nc.vector.tensor_copy(t[:], atB_ps[:])
                    atB.append(t)
                return atB

            htT_sb = small.tile([128, KT, BL], F32, tag="htT")

            def text_einsum(atB, kts):
                for kt in kts:
                    for b in range(BL):
                        scrt = work.tile([128, Tt], BF16, tag="scrt")
                        nc.vector.scalar_tensor_tensor(
                            out=scrt[:],
                            in0=tT8_sb[:, kt, b * Tt:(b + 1) * Tt],
                            scalar=1.0,
                            in1=atB[b][:],
                            op0=ALU.mult, op1=ALU.mult,
                            accum_out=htT_sb[:, kt, b:b + 1],
                        )

            # ---------- issue order (drives per-engine schedules) ----------
            pair_compute(0)
            sct_ps = text_matmuls()
            pair_compute(1)
            atB = text_softmax(sct_ps)
            pair_chain(0)
            pair_compute(2)
            text_einsum(atB, [0, 1])

            # wbs = 256*(h@Wb); hwhh = h@Whh + bh (true scale)
            wbs_ps = psG.tile([BL, H], F32, tag="psG", name="wbs")
            dr_gate(wbs_ps, hT8_sb, wb8_sb, True, True)
            wbst_sb = small.tile([BL, H], BF16, tag="wbst_sb")  # 256(hWb+bbt)
            nc.vector.scalar_tensor_tensor(
                out=wbst_sb[:], in0=wbs_ps[:], scalar=1.0, in1=bbtB,
                op0=ALU.mult, op1=ALU.add,
            )
            wbsv_sb = small.tile([BL, H], BF16, tag="wbsv_sb")  # 256(hWb+bbv)
            nc.vector.scalar_tensor_tensor(
                out=wbsv_sb[:], in0=wbs_ps[:], scalar=1.0, in1=bbvB,
                op0=ALU.mult, op1=ALU.add,
            )

            pair_chain(1)
            text_einsum(atB, [2, 3])
            pair_chain(2)
            pair3_batch(0)

            hwhh_ps = psG.tile([BL, H], F32, tag="psG", name="hwhh")
            for kt in range(KT):
                nc.tensor.matmul(
                    hwhh_ps[:], hTb_sb[:, kt, :], whh_sb[:, kt, :],
                    start=(kt == 0), stop=(kt == KT - 1), skip_group_check=True,
                )
            hwhh_sb = small.tile([BL, H], F32, tag="hwhh_sb")
            nc.vector.scalar_tensor_tensor(
                out=hwhh_sb[:], in0=hwhh_ps[:], scalar=1.0, in1=bhB,
                op0=ALU.mult, op1=ALU.add,
            )

            chain3_i(0)
            pair3_batch(1)

            # ---------- text gates (256x psums) ----------
            htT16 = small.tile([128, KT, 16], FP8, tag="htT16")
            nc.vector.tensor_scalar_mul(
                htT16[:, :, 0:BL], htT_sb[:, :, :], SW,
            )
            mt1_ps = psG.tile([BL, H], F32, tag="psG", name="mt1")
            nc.tensor.matmul(   # += 256*(h@Wb + bbt) via identity
                mt1_ps[:], eye_sb[0:BL, 0:BL], wbst_sb[:],
                start=True, stop=False, skip_group_check=True,
            )
            dr_gate(mt1_ps, htT16, vbt8_sb, False, True)
            mtv_t = small.tile([BL, H], F32, tag="mtv_t")
            nc.scalar.activation(mtv_t[:], mt1_ps[:], ACT.Tanh, scale=1.0 / SG)
            lgt_t = small.tile([BL, 1], F32, tag="lgt_t")
            scr8t = small.tile([BL, H], F32, tag="scr8t")
            nc.vector.scalar_tensor_tensor(
                out=scr8t[:], in0=mtv_t[:], scalar=1.0, in1=wbB_sb[:],
                op0=ALU.mult, op1=ALU.mult, accum_out=lgt_t[:],
            )
            ht2_ps = psG.tile([BL, H], F32, tag="psG", name="ht2")
            dr_gate(ht2_ps, htT16, wqe8_sb, True, True)
            ht2_sb = small.tile([BL, H], F32, tag="ht2_sb")
            nc.vector.scalar_tensor_tensor(
                out=ht2_sb[:], in0=ht2_ps[:], scalar=1.0, in1=bqeB,
                op0=ALU.mult, op1=ALU.add,
            )

            chain3_i(1)

            # ---------- frames gates + logits ----------
            rv8 = small.tile([BL, 1], F32, tag="rv8")
            nc.vector.reciprocal(rv8[:], sum8[:])
            nc.vector.tensor_scalar_mul(hv16n[:], hv16[:], rv8[:])
            for jt in range(KT):
                tp = psX.tile([128, BL], BF16, tag="psX", name=f"hvtp{jt}")
                nc.tensor.transpose(
                    tp[:], hv16n[0:BL, jt * 128:(jt + 1) * 128],
                    eye_sb[0:BL, 0:BL],
                )
                nc.vector.tensor_copy(hvT16[:, jt, 0:BL], tp[:])

            mv1_ps = psG.tile([BL, H], F32, tag="psG", name="mv1")
            nc.tensor.matmul(
                mv1_ps[:], eye_sb[0:BL, 0:BL], wbsv_sb[:],
                start=True, stop=False, skip_group_check=True,
            )
            dr_gate(mv1_ps, hvT16, vbv8_sb, False, True)
            mtv_v = small.tile([BL, H], F32, tag="mtv_v")
            nc.scalar.activation(mtv_v[:], mv1_ps[:], ACT.Tanh, scale=1.0 / SG)
            lgv_t = small.tile([BL, 1], F32, tag="lgv_t")
            scr8v = small.tile([BL, H], F32, tag="scr8v")
            nc.vector.scalar_tensor_tensor(
                out=scr8v[:], in0=mtv_v[:], scalar=1.0, in1=wbB_sb[:],
                op0=ALU.mult, op1=ALU.mult, accum_out=lgv_t[:],
            )

            # ---------- pre-reduced AllGather payload ----------
            # Each core ships (Zc, e0*m, e1*m): Zc = sum of its 16 logit
            # exps, e0/e1 masked to core 0. Post-AG work shrinks to one
            # fp32 matmul against a row selector (no exp / transpose on
            # the critical tail).
            ev = small.tile([BL, 1], BF16, tag="ev")
            et = small.tile([BL, 1], BF16, tag="et")
            nc.scalar.activation(ev[:], lgv_t[:], ACT.Exp)
            nc.scalar.activation(et[:], lgt_t[:], ACT.Exp)
            pay_ps = psE.tile([3, 1], F32, tag="psE", name="payps")
            nc.tensor.matmul(
                pay_ps[:], sel3_sb[0:BL, 0:3], ev[0:BL, 0:1],
                start=True, stop=False, skip_group_check=True,
            )
            nc.tensor.matmul(
                pay_ps[:], sel3_sb[0:BL, 3:6], et[0:BL, 0:1],
                start=False, stop=True, skip_group_check=True,
            )
            payload = small.tile([4, 1], F32, tag="payload")
            nc.vector.memset(payload[:], 0.0)
            nc.vector.tensor_copy(payload[0:3, 0:1], pay_ps[:, :])
            cc_in = dram.tile([4, 1], F32, tag="ccin")
            cc_out = dram.tile([4 * NC, 1], F32, tag="ccout",
                               addr_space="Shared")
            nc.sync.dma_start(cc_in[0:4], payload[:])
            nc.gpsimd.collective_compute(
                "AllGather", ALU.bypass,
                replica_groups=[list(range(NC))],
                ins=[cc_in[:].opt()],
                outs=[cc_out[:].opt()],
            )

            # overlap the AG: hv2 = 256*(hv@Wve.T + bve)
            hv2_ps = psG.tile([BL, H], F32, tag="psG", name="hv2")
            dr_gate(hv2_ps, hvT16, wve8_sb, True, True)
            hv2_sb = small.tile([BL, H], F32, tag="hv2_sb")
            nc.vector.scalar_tensor_tensor(
                out=hv2_sb[:], in0=hv2_ps[:], scalar=1.0, in1=bveB,
                op0=ALU.mult, op1=ALU.add,
            )

            # ---------- global beta from the gathered stats ----------
            g_col = small.tile([4 * NC, 1], F32, tag="g")
            nc.sync.dma_start(g_col[:, :], cc_out[:, :])
            zrow_ps = psB.tile([1, 3], F32, tag="psB", name="zrow")
            nc.tensor.matmul(   # [Z, e0, e1] row = g_col.T @ sel32
                zrow_ps[:], g_col[0:4 * NC, 0:1], sel32_sb[0:4 * NC, 0:3],
                start=True, stop=True, skip_group_check=True,
            )
            rg = small.tile([1, 1], F32, tag="rg")
            nc.vector.reciprocal(rg[:], zrow_ps[:, 0:1])
            betas = small.tile([1, 2], BF16, tag="betas")
            nc.vector.tensor_scalar_mul(betas[:], zrow_ps[:, 1:3], rg[:])
            beta8_ps = psB.tile([BL, 2], F32, tag="psB", name="beta8")
            nc.tensor.matmul(   # beta/256 broadcast to 8 partitions
                beta8_ps[:], oq_sb[0:1, 0:BL], betas[0:1, 0:2],
                start=True, stop=True,
            )
            # ---------- out = tanh(hWhh+bh + b0*hv2 + b1*ht2) ----------
            t1 = small.tile([BL, H], F32, tag="t1")
            nc.vector.scalar_tensor_tensor(
                out=t1[:], in0=hv2_sb[:], scalar=beta8_ps[:, 0:1], in1=hwhh_sb[:],
                op0=ALU.mult, op1=ALU.add,
            )
            s1 = small.tile([BL, H], F32, tag="s1")
            nc.vector.scalar_tensor_tensor(
                out=s1[:], in0=ht2_sb[:], scalar=beta8_ps[:, 1:2], in1=t1[:],
                op0=ALU.mult, op1=ALU.add,
            )
            out_sb = small.tile([BL, H], F32, tag="out_sb")
            nc.scalar.activation(out_sb[:], s1[:], ACT.Tanh)
            nc.sync.dma_start(out_ext, out_sb[:])

    nc.compile()
    return nc


_cached_nc = None


def _get_nc():
    global _cached_nc
    if _cached_nc is None:
        _cached_nc = build_nc()
    return _cached_nc


def _bf(a):
    return np.asarray(a, np.float32).astype(NP_BF16)


def _f8(a):
    return np.asarray(a, np.float32).astype(NP_FP8)


def _pack_w(w):
    """[512,512] -> [128, 4*512] with free = kt*512 + j, k = kt*128 + p."""
    return np.ascontiguousarray(
        np.asarray(w, np.float32).reshape(KT, 128, H).transpose(1, 0, 2)
        .reshape(128, KT * H)
    )


def make_in_maps(inputs):
    h = np.asarray(inputs["h"], np.float32)
    frames = np.asarray(inputs["hidden_frames"], np.float32)
    text = np.asarray(inputs["hidden_text"], np.float32)

    Vav = np.asarray(inputs["Vav"], np.float32)
    Vat = np.asarray(inputs["Vat"], np.float32)
    wb = np.asarray(inputs["wb"], np.float32)

    # vavZ8[p, jtp, q, 2v+c] = 16*Vav[(2jtp+q)*128+p] if c == v (pad to 16)
    vavZ = np.zeros((128, 2, 2, 16), np.float32)
    for jtp in range(2):
        for q in range(2):
            col = Vav[(2 * jtp + q) * 128:(2 * jtp + q + 1) * 128] * SW
            for v in range(2):
                vavZ[:, jtp, q, 2 * v + v] = col

    biasr = np.zeros((1, 8 * H), np.float32)
    biasr = _bf(biasr)
    biasB = np.zeros((BL, 7 * H), np.float32)
    scales = [SW, SW, SG, SG, SG, SG, 1.0]
    for i, (k, s) in enumerate(zip(
            ["bav", "bat", "bve", "bqe", "bbv", "bbt", "bh"], scales)):
        biasB[:, i * H:(i + 1) * H] = np.asarray(inputs[k], np.float32)[None, :] * s
    biasB = _bf(biasB)
    wbB = np.ascontiguousarray(np.broadcast_to(wb, (BL, H))).astype(np.float32)
    blkI = np.zeros((BL, BL, Tt), np.float32)
    for b in range(BL):
        blkI[b, b, :] = 1.0
    blkI = _bf(blkI.reshape(BL, BL * Tt))

    hTp = np.zeros((128, KT, 16), np.float32)   # per-core filled below
    f8e_shared = np.zeros((128, 128 + KT * H), np.float32)
    f8e_shared[:, 64:128] = vavZ.reshape(128, 64)
    f8e_shared[:, 128:] = _pack_w(inputs["Uav"]) * SW
    wav8 = _f8(_pack_w(inputs["Wav"]) * SW)
    uat8 = _f8(_pack_w(inputs["Uat"]) * SW)

    f8t = np.zeros((128, 2 * KT * H), np.float32)
    # tT8 filled per-core below
    f8t[:, KT * H:] = _pack_w(inputs["Wat"]) * SW

    f8g = np.zeros((128, 5 * KT * H), np.float32)
    f8g[:, 0:KT * H] = _pack_w(inputs["Wb"]) * SG
    f8g[:, KT * H:2 * KT * H] = _pack_w(inputs["Vbt"]) * SW
    f8g[:, 2 * KT * H:3 * KT * H] = _pack_w(np.asarray(inputs["Wqe"], np.float32).T) * SW
    f8g[:, 3 * KT * H:4 * KT * H] = _pack_w(inputs["Vbv"]) * SW
    f8g[:, 4 * KT * H:] = _pack_w(np.asarray(inputs["Wve"], np.float32).T) * SW
    f8g = _f8(f8g)

    sel32 = np.zeros((4 * NC, 3), np.float32)
    for k in range(NC):
        sel32[4 * k + 0, 0] = 1.0
        sel32[4 * k + 1, 1] = 1.0
        sel32[4 * k + 2, 2] = 1.0
    shared = dict(
        sel32=sel32,
        f8g=f8g, wav8=wav8, uat8=uat8,
        whh_bf=_bf(_pack_w(inputs["Whh"])),
        biasr=biasr, biasB=biasB, wbB=wbB, blkI=blkI,
    )

    in_maps = []
    for i in range(NC):
        sl = slice(i * BL, (i + 1) * BL)
        fTc = np.ascontiguousarray(
            frames[sl].transpose(0, 2, 1)       # [BL, H, Tv]
            .reshape(BL, KT, 128, Tv)
            .transpose(0, 2, 1, 3)              # [BL, 128, KT, Tv]
            .reshape(BL, 128, KT * Tv)
        )
        fRc = np.ascontiguousarray(
            frames[sl]                          # [BL, Tv, H]
            .reshape(BL, KT, 128, H)
            .transpose(0, 2, 1, 3)              # [BL, 128, TT, H]
            .reshape(BL, 128, KT * H)
        )
        tTc = np.ascontiguousarray(
            text[sl].transpose(2, 0, 1)         # [H, BL, Tt]
            .reshape(KT, 128, BL, Tt)
            .transpose(1, 0, 2, 3)              # [128, KT, BL, Tt]
            .reshape(128, KT * BL * Tt)
        )
        hTc = np.ascontiguousarray(
            h[sl].T.reshape(KT, 128, BL).transpose(1, 0, 2).reshape(128, KT * BL)
        )
        f8e = f8e_shared.copy()
        f8e[:, 0:64] = 0.0
        f8e[:, 0:64].reshape(128, KT, 16)[:, :, 0:BL] = hTc.reshape(128, KT, BL)
        f8t_i = f8t.copy()
        f8t_i[:, 0:KT * H] = tTc
        bfs = np.zeros((128, 128 + KT * BL + KT), np.float32)
        bfs[:, 0:128] = np.eye(128, dtype=np.float32)
        bfs[:, 128:128 + KT * BL] = hTc
        bfs[:, 128 + KT * BL:] = np.ascontiguousarray(
            np.asarray(inputs["Vat"], np.float32).reshape(KT, 128).T)
        sel3 = np.zeros((BL, 6), np.float32)
        sel3[:, 0] = 1.0
        sel3[:, 3] = 1.0
        if i == 0:
            sel3[0, 1] = 1.0
            sel3[1, 2] = 1.0
        in_maps.append(dict(
            shared,
            fT8=_f8(fTc), fR8=_f8(fRc),
            f8e=_f8(f8e), f8t=_f8(f8t_i), bfs=_bf(bfs),
            sel3=_bf(sel3),
        ))
    return in_maps


def run(inputs, trace=False, **kw):
    nc = _get_nc()
    in_maps = make_in_maps(inputs)
    res = run_bass_kernel_spmd(nc, in_maps, core_ids=list(range(NC)), trace=trace, **kw)
    out = np.concatenate([res.results[i]["out"] for i in range(NC)], axis=0)
    return out, res


def kernel(**inputs) -> np.ndarray:
    out, _ = run(inputs, trace=False)
    return out



# revision 21
# speedup vs baseline: 1.1449x; 1.0454x over previous
"""Trainium2 Bass kernel for nn_AttentionTwoStream (two-stream Bahdanau attention
with global gating softmax), distributed over 8 NeuronCores.

Sharding: data-parallel over batch B=64 -> 8 batches per core; weights
replicated. One 16-float-per-core AllGather feeds the global beta softmax
(preceded by a warmup AllGather that hides the ncfw pipeline spin-up).

Compute strategy:
 - frames matmul, score dot and attention einsum run on the TensorEngine in
   fp8e4 DoubleRow mode (2 k-rows per PE pass). Weights that land in fp8 are
   prescaled by 16 (or 256 for gate weights) host-side to clear the fp8
   subnormal floor; descales fold into ACT scale params / the beta scalar.
 - The attention einsum uses exp-unnormalized weights (values ~1.0, fp8-safe)
   against a second [t,h]-layout copy of frames; the softmax divide folds into
   the per-batch evacuation op.
 - gate biases ride the Vector evacuation STTs (no bias matmuls); tanh/exp on
   Scalar; fp32 PSUM accumulation everywhere.
 - DMAs: blobbed by dtype/criticality; sync hwdge queue carries the critical
   score-path set first, scalar/gpsimd queues are gated behind fT[1] so the
   frames stream gets the HBM bandwidth first (per-queue bw is the limit).

Self-contained: hardcodes shapes B=64, Tv=512, Tt=64, H=512, 8 cores.
"""

import numpy as np
import ml_dtypes

import concourse.bacc as bacc
import concourse.mybir as mybir
import concourse.tile as tile
from concourse.bass_utils import run_bass_kernel_spmd

NC = 8          # cores
B = 64          # global batch
BL = B // NC    # batches per core = 8
H = 512
Tv = 512
Tt = 64
KT = H // 128   # 4 contraction tiles (and 4 Tv partition tiles)
F32 = mybir.dt.float32
BF16 = mybir.dt.bfloat16
FP8 = mybir.dt.float8e4
NP_BF16 = ml_dtypes.bfloat16
NP_FP8 = ml_dtypes.float8_e4m3

SW = 16.0       # fp8 prescale for score-path weights
SG = 256.0      # fp8 prescale product for gate/logit-path psums
DR = mybir.MatmulPerfMode.DoubleRow


def build_nc():
    nc = bacc.Bacc(
        "TRN2", target_bir_lowering=False, debug=False,
        enable_asserts=False, num_devices=NC, num_swdge_queues=4,
    )

    def inp(name, shape, dt):
        return nc.dram_tensor(name, list(shape), dt, kind="ExternalInput").ap()

    # --- external inputs (per-core shards; layouts match SBUF tiles) ---
    # fp8 early blob: hT8(64) | vavZ8(64) | uav8 (cols)
    f8e = inp("f8e", (128, 128 + KT * H), FP8)
    wav8 = inp("wav8", (128, KT * H), FP8)
    uat8 = inp("uat8", (128, KT * H), FP8)
    # bf16 smalls blob: eye(128) | hTb(KT*BL) | vatT(KT)
    bfs = inp("bfs", (128, 128 + KT * BL + KT), BF16)
    biasr = inp("biasr", (1, 8 * H), BF16)       # scaled bias rows (warmup fodder)
    biasB = inp("biasB", (BL, 7 * H), BF16)      # per-gate bias rows bcast to 8 parts
    blkI = inp("blkI", (BL, BL * Tt), BF16)      # blkI[b, b'*64+t] = (b==b')
    wbB = inp("wbB", (BL, H), F32)               # wb broadcast to 8 partitions
    fT8 = inp("fT8", (BL, 128, KT * Tv), FP8)    # frames^T [b][p][kt*Tv+t]
    fR8 = inp("fR8", (BL, 128, KT * H), FP8)     # frames   [b][p][tt*H+h], t=tt*128+p
    # fp8 text blob: tT8 | wat8
    f8t = inp("f8t", (128, 2 * KT * H), FP8)
    whh_bf = inp("whh_bf", (128, KT * H), BF16)  # Whh (true scale)
    # fp8 late gate blob: wb8(256x) | vbt8 | wqe8 | vbv8 | wve8 (16x)
    f8g = inp("f8g", (128, 5 * KT * H), FP8)
    sel3 = inp("sel3", (BL, 6), BF16)     # payload selectors (core-0 mask)
    sel32 = inp("sel32", (4 * NC, 3), F32)  # post-AG row selector
    out_ext = nc.dram_tensor("out", [BL, H], F32, kind="ExternalOutput").ap()

    ACT = mybir.ActivationFunctionType
    ALU = mybir.AluOpType

    with tile.TileContext(nc) as tc:
        with (
            tc.tile_pool(name="wres", bufs=1) as wres,
            tc.tile_pool(name="work", bufs=4) as work,
            tc.tile_pool(name="small", bufs=1) as small,
            tc.tile_pool(name="psX", bufs=3, space="PSUM") as psX,
            tc.tile_pool(name="psS", bufs=2, space="PSUM") as psS,
            tc.tile_pool(name="psB", bufs=1, space="PSUM") as psB,
            tc.tile_pool(name="psE", bufs=1, space="PSUM") as psE,
            tc.tile_pool(name="psG", bufs=1, space="PSUM") as psG,
            tc.tile_pool(name="dram", bufs=1, space="DRAM") as dram,
        ):
            def load(pool, ap_in, shape, tag, dt, engine, name=None):
                t = pool.tile(list(shape), dt, tag=tag, name=name or tag)
                engine.dma_start(t[:], ap_in)
                return t

            # ---------- warmup collective (primes ncfw; result unused) ----
            warm_out = dram.tile([4 * NC, 1], F32, tag="warmout", addr_space="Shared")
            warm_in = dram.tile([4, 1], F32, tag="warmin")
            nc.gpsimd.collective_compute(
                "AllGather", ALU.bypass,
                replica_groups=[list(range(NC))],
                ins=[warm_in[:].opt()],
                outs=[warm_out[:].opt()],
            )

            # ---------- DMAs ----------
            # pair0 critical set spread across all three queues in parallel:
            # gpsimd pulls fT0/fT1 (4 swdge sub-queues), scalar pulls the
            # P1 fp8 blob, sync pulls wav8 + smalls. Bulk streams follow,
            # gated behind fT1 where they would steal critical bandwidth.
            # sync hwdge:
            biasr_sb = load(wres, biasr, (1, 8 * H), "biasr", BF16, nc.sync)
            wav8_t = load(wres, wav8, (128, KT, H), "wav8", FP8, nc.sync)
            wav8_sb = wav8_t[:, :, :]
            bfs_sb = load(wres, bfs, (128, 128 + KT * BL + KT), "bfs", BF16, nc.sync)
            eye_sb = bfs_sb[:, 0:128]
            hTb_sb = bfs_sb[:, 128:128 + KT * BL].rearrange("p (k b) -> p k b", k=KT)
            vatT_sb = bfs_sb[:, 128 + KT * BL:]
            blkI_sb = load(wres, blkI, (BL, BL * Tt), "blkI", BF16, nc.sync)
            biasB_sb = load(wres, biasB, (BL, 7 * H), "biasB", BF16, nc.sync)
            fT_sb = [None] * BL
            for b in range(2, BL):
                t = wres.tile([128, KT, Tv], FP8, tag=f"fT{b}", name=f"fTs{b}")
                nc.sync.dma_start(t[:], fT8[b])
                fT_sb[b] = t
            wbB_sb = load(wres, wbB, (BL, H), "wbB", F32, nc.sync)
            sel3_sb = load(small, sel3, (BL, 6), "sel3", BF16, nc.sync)
            sel32_sb = load(small, sel32, (4 * NC, 3), "sel32", F32, nc.sync)
            # gpsimd swdge: f8e + fT0/fT1 first (parallel sub-queues), gated bulk
            f8e_sb = load(wres, f8e, (128, 128 + KT * H), "f8e", FP8, nc.gpsimd)
            hT8_sb = f8e_sb[:, 0:64].rearrange("p (k s) -> p k s", k=KT)
            vavZ8_sb = f8e_sb[:, 64:128].rearrange("p (a b c) -> p a b c", a=2, b=2)
            uav8_sb = f8e_sb[:, 128:].rearrange("p (k n) -> p k n", k=KT)
            for b in range(2):
                t = wres.tile([128, KT, Tv], FP8, tag=f"fT{b}", name=f"fTs{b}")
                nc.gpsimd.dma_start(t[:], fT8[b])
                fT_sb[b] = t
            qgate_g = small.tile([1, 2], FP8, tag="qgate_g")
            nc.gpsimd.tensor_copy(qgate_g[:], fT_sb[1][0:1, 0, 0:2])
            fR_sb = [None] * BL
            for b in range(4):
                t = wres.tile([128, KT, H], FP8, tag=f"fR{b}", name=f"fRs{b}")
                nc.gpsimd.dma_start(t[:], fR8[b])
                fR_sb[b] = t
            f8g_sb = load(wres, f8g, (128, 5 * KT * H), "f8g", FP8, nc.gpsimd)
            wb8_sb = f8g_sb[:, 0:KT * H].rearrange("p (k n) -> p k n", k=KT)
            vbt8_sb = f8g_sb[:, KT * H:2 * KT * H].rearrange("p (k n) -> p k n", k=KT)
            wqe8_sb = f8g_sb[:, 2 * KT * H:3 * KT * H].rearrange("p (k n) -> p k n", k=KT)
            for b in range(4, BL):
                t = wres.tile([128, KT, H], FP8, tag=f"fR{b}", name=f"fRs{b}")
                nc.gpsimd.dma_start(t[:], fR8[b])
                fR_sb[b] = t
            vbv8_sb = f8g_sb[:, 3 * KT * H:4 * KT * H].rearrange("p (k n) -> p k n", k=KT)
            wve8_sb = f8g_sb[:, 4 * KT * H:].rearrange("p (k n) -> p k n", k=KT)
            # scalar hwdge: uat8 first, gated text/late weights
            uat8_t = load(wres, uat8, (128, KT, H), "uat8", FP8, nc.scalar)
            uat8_sb = uat8_t[:, :, :]
            qgate_s = small.tile([1, 2], FP8, tag="qgate_s")
            nc.scalar.copy(qgate_s[:], fT_sb[1][0:1, 0, 0:2])
            f8t_sb = load(wres, f8t, (128, 2 * KT * H), "f8t", FP8, nc.scalar)
            tT8_sb = f8t_sb[:, 0:KT * H].rearrange("p (k n) -> p k n", k=KT)
            wat8_sb = f8t_sb[:, KT * H:].rearrange("p (k n) -> p k n", k=KT)
            whh_sb = load(wres, whh_bf, (128, KT, H), "whh", BF16, nc.scalar)

            ones_sb = small.tile([1, 128], BF16, tag="ones")
            nc.vector.memset(ones_sb[:], 1.0)
            oq_sb = small.tile([1, BL], BF16, tag="oq")
            nc.vector.memset(oq_sb[:], 1.0 / SG)
            onesC_sb = small.tile([128, 1], BF16, tag="onesC")
            nc.vector.memset(onesC_sb[:], 1.0)

            def bB(i):
                return biasB_sb[:, i * H:(i + 1) * H]
            # cols: 16bav, 16bat, 256bve, 256bqe, 256bbv, 256bbt, bh
            bavB, batB, bveB, bqeB, bbvB, bbtB, bhB = (bB(i) for i in range(7))

            # PE warmup: junk matmuls on the first-arriving bytes (p-state
            # ramp while the frames stream loads)
            warm_ps = psX.tile([128, Tv], F32, tag="psX", name="warmps")
            for w in range(6):
                nc.tensor.matmul(
                    warm_ps[0:BL, :], ones_sb[0:1, 0:BL], biasr_sb[0:1, 0:H],
                    start=True, stop=True, skip_group_check=True,
                )

            # fp8 DoubleRow gate: psum[8,H] += (hT|hvT|htT).T @ W  (K=512 via
            # 2 DoubleRow passes), optional extra rows merged by caller.
            def dr_gate(ps, lhs_sb, w_sb, start, stop):
                for ktp in range(2):
                    nc.tensor.matmul(
                        ps[:], lhs_sb[:, 2 * ktp:2 * ktp + 2, 0:BL],
                        w_sb[:, 2 * ktp:2 * ktp + 2, :],
                        start=(start and ktp == 0), stop=(stop and ktp == 1),
                        perf_mode=DR, skip_group_check=True,
                    )

            # ---------- P1: h-projections (score biases) ----------
            uhvb_ps = psG.tile([BL, H], F32, tag="psG", name="uhvb")
            dr_gate(uhvb_ps, hT8_sb, uav8_sb, True, True)
            uhvb_s = small.tile([BL, H], BF16, tag="uhvb_s")
            # uhvb = (16*hUav)/16 + bav
            nc.vector.scalar_tensor_tensor(
                out=uhvb_s[:], in0=uhvb_ps[:], scalar=1.0 / SW, in1=bavB,
                op0=ALU.mult, op1=ALU.add,
            )

            # frames bias in [512,8] layout for per-partition ACT bias
            uhvbT_sb = small.tile([128, KT * BL], F32, tag="uhvbT")
            for jt in range(KT):
                tp = psB.tile([128, BL], BF16, tag="psB", name=f"tpv{jt}")
                nc.tensor.transpose(
                    tp[:], uhvb_s[0:BL, jt * 128:(jt + 1) * 128],
                    eye_sb[0:BL, 0:BL],
                )
                nc.vector.tensor_copy(uhvbT_sb[:, jt * BL:(jt + 1) * BL], tp[:])

            uhtb_ps = psG.tile([BL, H], F32, tag="psG", name="uhtb")
            dr_gate(uhtb_ps, hT8_sb, uat8_sb, True, True)
            uhtb_s = small.tile([BL, H], BF16, tag="uhtb_s")   # 16x scale
            nc.vector.scalar_tensor_tensor(
                out=uhtb_s[:], in0=uhtb_ps[:], scalar=1.0, in1=batB,
                op0=ALU.mult, op1=ALU.add,
            )

            # ---------- frames pair machinery ----------
            NP = BL // 2
            hv16 = small.tile([BL, H], BF16, tag="hv16")
            hv16n = small.tile([BL, H], BF16, tag="hv16n")
            sum8 = small.tile([BL, 1], F32, tag="sum8")
            hvT16 = small.tile([128, KT, 16], FP8, tag="hvT16")
            yv_tiles = {}
            scv_tiles = {}
            avT_tiles = {}
            hv16row = small.tile([1, BL * H], BF16, tag="hv16row")

            def pair_compute(g):
                """xps matmuls + tanh + score accumulation for pair g."""
                scv_g = psS.tile([2, Tv], F32, tag="scS", name=f"scv{g}")
                scv_tiles[g] = scv_g
                nmm = 0
                for jtp in range(2):
                    for i in range(2):
                        b = 2 * g + i
                        yv = work.tile([128, 2, Tv], FP8, tag="yv",
                                       name=f"yv{g}_{jtp}_{i}", bufs=6)
                        for q in range(2):
                            jt = 2 * jtp + q
                            xps = psX.tile([128, Tv], F32, tag="psX",
                                           name=f"xps{g}_{jtp}_{i}_{q}")
                            for ktp in range(2):
                                nc.tensor.matmul(
                                    xps[:],
                                    wav8_sb[:, 2 * ktp:2 * ktp + 2,
                                            jt * 128:(jt + 1) * 128],
                                    fT_sb[b][:, 2 * ktp:2 * ktp + 2, :],
                                    start=(ktp == 0), stop=(ktp == 1),
                                    perf_mode=DR, skip_group_check=True,
                                )
                            # yv = tanh(xps/16 + Uhv + bav)
                            nc.scalar.activation(
                                yv[:, q, :], xps[:], ACT.Tanh,
                                scale=1.0 / SW,
                                bias=uhvbT_sb[:, jt * BL + b: jt * BL + b + 1],
                            )
                        yv_tiles[(g, jtp, i)] = yv
                        nmm += 1
                        nc.tensor.matmul(    # scv += (16Vav).T @ yv -> 16*s
                            scv_g[:],
                            vavZ8_sb[:, jtp, :, 2 * i:2 * i + 2],
                            yv[:, :, :],
                            start=(nmm == 1), stop=(nmm == 4),
                            perf_mode=DR, skip_group_check=True,
                        )

            def pair_chain(g):
                """exp -> transpose weights -> PE einsum -> evac for pair g.
                Normalization is deferred: evacs write 16*unnormalized rows;
                sumv is shipped to sum8 for one post-assembly divide."""
                last = (g == NP - 1)
                tpool, epool = (psX, psX) if last else (psB, psE)
                scv_g = scv_tiles[g]
                avp = small.tile([2, Tv], BF16, tag="avp", name=f"avp{g}", bufs=2)
                sumv = small.tile([2, 1], F32, tag="sumv", name=f"sumv{g}", bufs=2)
                nc.scalar.activation(
                    avp[:], scv_g[:], ACT.Exp, scale=1.0 / SW,
                    accum_out=sumv[:],
                )
                nc.sync.dma_start(sum8[2 * g:2 * g + 2, 0:1], sumv[:, :])
                avT = small.tile([128, KT, 16], FP8, tag="avT", name=f"avT{g}", bufs=2)
                avT_tiles[g] = avT
                for tt in range(KT):
                    tp = tpool.tile([128, 2], BF16, tag=tpool.name, name=f"avtp{g}_{tt}")
                    nc.tensor.transpose(
                        tp[:], avp[0:2, tt * 128:(tt + 1) * 128],
                        eye_sb[0:2, 0:2],
                    )
                    nc.vector.tensor_copy(avT[:, tt, 0:2], tp[:])
                for i in range(2):
                    b = 2 * g + i
                    eps = epool.tile([1, H], F32, tag=epool.name, name=f"eps{g}_{i}")
                    for ttp in range(2):
                        nc.tensor.matmul(
                            eps[:],
                            avT[:, 2 * ttp:2 * ttp + 2, i:i + 1],
                            fR_sb[b][:, 2 * ttp:2 * ttp + 2, :],
                            start=(ttp == 0), stop=(ttp == 1),
                            perf_mode=DR, skip_group_check=True,
                        )
                    # hv16row[b] = 16 * unnormalized weighted sum
                    nc.vector.tensor_scalar_mul(
                        hv16row[0:1, b * H:(b + 1) * H], eps[:], SW,
                    )
                nc.sync.dma_start(
                    hv16[2 * g:2 * g + 2, :],
                    hv16row[0:1, 2 * g * H:(2 * g + 2) * H],
                )

            # --- pair 3 split per-batch: batch 6's chain overlaps batch 7 ---
            scv3 = {}
            avT3 = small.tile([128, KT, 16], FP8, tag="avT", name="avT3", bufs=2)

            def pair3_batch(i):
                b = 6 + i
                scv_b = psS.tile([1, Tv], F32, tag="scS", name=f"scv3_{i}")
                scv3[i] = scv_b
                nmm = 0
                for jtp in range(2):
                    yv = work.tile([128, 2, Tv], FP8, tag="yv",
                                   name=f"yv3_{jtp}_{i}", bufs=6)
                    for q in range(2):
                        jt = 2 * jtp + q
                        xps = psX.tile([128, Tv], F32, tag="psX",
                                       name=f"xps3_{jtp}_{i}_{q}")
                        for ktp in range(2):
                            nc.tensor.matmul(
                                xps[:],
                                wav8_sb[:, 2 * ktp:2 * ktp + 2,
                                        jt * 128:(jt + 1) * 128],
                                fT_sb[b][:, 2 * ktp:2 * ktp + 2, :],
                                start=(ktp == 0), stop=(ktp == 1),
                                perf_mode=DR, skip_group_check=True,
                            )
                        nc.scalar.activation(
                            yv[:, q, :], xps[:], ACT.Tanh,
                            scale=1.0 / SW,
                            bias=uhvbT_sb[:, jt * BL + b: jt * BL + b + 1],
                        )
                    nmm += 1
                    nc.tensor.matmul(   # single-batch score row
                        scv_b[:],
                        vavZ8_sb[:, jtp, :, 0:1],
                        yv[:, :, :],
                        start=(nmm == 1), stop=(nmm == 2),
                        perf_mode=DR, skip_group_check=True,
                    )

            def chain3_i(i):
                # i==0 runs concurrently with batch 7's xps matmuls -> keep it
                # off the psX banks; i==1 runs after all xps -> psX is idle.
                tpool, epool = (psX, psX) if i == 1 else (psB, psE)
                b = 6 + i
                avp = small.tile([1, Tv], BF16, tag="avp", name=f"avp3_{i}", bufs=2)
                sumv = small.tile([1, 1], F32, tag="sumv", name=f"sumv3_{i}", bufs=2)
                nc.scalar.activation(
                    avp[:], scv3[i][:], ACT.Exp, scale=1.0 / SW,
                    accum_out=sumv[:],
                )
                nc.sync.dma_start(sum8[b:b + 1, 0:1], sumv[:, :])
                for tt in range(KT):
                    tp = tpool.tile([128, 1], BF16, tag=tpool.name, name=f"avtp3_{i}_{tt}")
                    nc.tensor.transpose(
                        tp[:], avp[0:1, tt * 128:(tt + 1) * 128],
                        eye_sb[0:1, 0:1],
                    )
                    nc.vector.tensor_copy(avT3[:, tt, i:i + 1], tp[:])
                eps = epool.tile([1, H], F32, tag=epool.name, name=f"eps3_{i}")
                for ttp in range(2):
                    nc.tensor.matmul(
                        eps[:],
                        avT3[:, 2 * ttp:2 * ttp + 2, i:i + 1],
                        fR_sb[b][:, 2 * ttp:2 * ttp + 2, :],
                        start=(ttp == 0), stop=(ttp == 1),
                        perf_mode=DR, skip_group_check=True,
                    )
                nc.vector.tensor_scalar_mul(
                    hv16row[0:1, b * H:(b + 1) * H], eps[:], SW,
                )
                nc.sync.dma_start(hv16[b:b + 1, :], hv16row[0:1, b * H:(b + 1) * H])

            # ---------- text stream pieces ----------
            def text_matmuls():
                sct_ps = psG.tile([1, BL * Tt], F32, tag="psG", name="sct")
                pend = []

                def flush_sct():
                    for yt_, jt_ in pend:
                        nc.tensor.matmul(
                            sct_ps[:], vatT_sb[:, jt_: jt_ + 1], yt_[:],
                            start=(jt_ == 0), stop=(jt_ == KT - 1),
                            skip_group_check=True,
                        )
                    pend.clear()

                for jt in range(KT):
                    xt_ps = psX.tile([128, BL * Tt], F32, tag="psX", name=f"xt{jt}")
                    for ktp in range(2):
                        nc.tensor.matmul(
                            xt_ps[:],
                            wat8_sb[:, 2 * ktp:2 * ktp + 2, jt * 128:(jt + 1) * 128],
                            tT8_sb[:, 2 * ktp:2 * ktp + 2, :],
                            start=(ktp == 0), stop=False,
                            perf_mode=DR, skip_group_check=True,
                        )
                    nc.tensor.matmul(   # bias: += 16*Uhtb[b, jt*128+j] via blkI
                        xt_ps[:], uhtb_s[0:BL, jt * 128:(jt + 1) * 128], blkI_sb[:],
                        start=False, stop=True, skip_group_check=True,
                    )
                    flush_sct()
                    yt = work.tile([128, BL * Tt], BF16, tag="yt", name=f"yt{jt}")
                    nc.scalar.activation(yt[:], xt_ps[:], ACT.Tanh, scale=1.0 / SW)
                    pend.append((yt, jt))
                flush_sct()
                return sct_ps

            def text_softmax(sct_ps):
                sct_sb = small.tile([1, BL * Tt], F32, tag="sct_sb")
                nc.vector.tensor_copy(sct_sb[:], sct_ps[:])
                st8 = small.tile([BL, Tt], F32, tag="st8")
                nc.sync.dma_start(st8[:, :], sct_sb[0:1, :])
                expt = small.tile([BL, Tt], F32, tag="expt")
                sumt = small.tile([BL, 1], F32, tag="sumt")
                nc.scalar.activation(expt[:], st8[:], ACT.Exp, accum_out=sumt[:])
                rt = small.tile([BL, 1], F32, tag="rt")
                nc.vector.reciprocal(rt[:], sumt[:])
                at_sb = small.tile([BL, Tt], BF16, tag="at")
                nc.vector.tensor_scalar_mul(at_sb[:], expt[:], rt[:])
                atRows = small.tile([1, BL * Tt], BF16, tag="atRows")
                nc.sync.dma_start(atRows[0:1, :], at_sb[:, :])
                atB = []
                for b in range(BL):
                    atB_ps = psB.tile([128, Tt], F32, tag="psB", name=f"atB{b}")
                    src = at_sb[0:1, :] if b == 0 else atRows[0:1, b * Tt:(b + 1) * Tt]
                    nc.tensor.matmul(
                        atB_ps[:], ones_sb[0:1, 0:128], src,
                        start=True, stop=True,
                    )
                    t = work.tile([128, Tt], BF16, tag="atB_sb", name=f"atBs{b}", bufs=8)
                    nc.vector.tensor_copy(t[:], atB_ps[:])
                    atB.append(t)
                return atB

            htT_sb = small.tile([128, KT, BL], F32, tag="htT")

            def text_einsum(atB, kts):
                for kt in kts:
                    for b in range(BL):
                        scrt = work.tile([128, Tt], BF16, tag="scrt")
                        nc.vector.scalar_tensor_tensor(
                            out=scrt[:],
                            in0=tT8_sb[:, kt, b * Tt:(b + 1) * Tt],
                            scalar=1.0,
                            in1=atB[b][:],
                            op0=ALU.mult, op1=ALU.mult,
                            accum_out=htT_sb[:, kt, b:b + 1],
                        )

            # ---------- issue order (drives per-engine schedules) ----------
            pair_compute(0)
            sct_ps = text_matmuls()
            pair_compute(1)
            atB = text_softmax(sct_ps)
            pair_chain(0)
            pair_compute(2)
            text_einsum(atB, [0, 1])

            # wbs = 256*(h@Wb); hwhh = h@Whh + bh (true scale)
            wbs_ps = psG.tile([BL, H], F32, tag="psG", name="wbs")
            dr_gate(wbs_ps, hT8_sb, wb8_sb, True, True)
            wbst_sb = small.tile([BL, H], BF16, tag="wbst_sb")  # 256(hWb+bbt)
            nc.vector.scalar_tensor_tensor(
                out=wbst_sb[:], in0=wbs_ps[:], scalar=1.0, in1=bbtB,
                op0=ALU.mult, op1=ALU.add,
            )
            wbsv_sb = small.tile([BL, H], BF16, tag="wbsv_sb")  # 256(hWb+bbv)
            nc.vector.scalar_tensor_tensor(
                out=wbsv_sb[:], in0=wbs_ps[:], scalar=1.0, in1=bbvB,
                op0=ALU.mult, op1=ALU.add,
            )

            pair_chain(1)
            text_einsum(atB, [2, 3])
            pair_chain(2)
            pair3_batch(0)

            hwhh_ps = psG.tile([BL, H], F32, tag="psG", name="hwhh")
            for kt in range(KT):
                nc.tensor.matmul(
                    hwhh_ps[:], hTb_sb[:, kt, :], whh_sb[:, kt, :],
                    start=(kt == 0), stop=(kt == KT - 1), skip_group_check=True,
                )
            hwhh_sb = small.tile([BL, H], F32, tag="hwhh_sb")
            nc.vector.scalar_tensor_tensor(
                out=hwhh_sb[:], in0=hwhh_ps[:], scalar=1.0, in1=bhB,
                op0=ALU.mult, op1=ALU.add,
            )

            chain3_i(0)
            pair3_batch(1)

            # ---------- text gates (256x psums) ----------
            htT16 = small.tile([128, KT, 16], FP8, tag="htT16")
            nc.vector.tensor_scalar_mul(
                htT16[:, :, 0:BL], htT_sb[:, :, :], SW,
            )
            mt1_ps = psG.tile([BL, H], F32, tag="psG", name="mt1")
            nc.tensor.matmul(   # += 256*(h@Wb + bbt) via identity
                mt1_ps[:], eye_sb[0:BL, 0:BL], wbst_sb[:],
                start=True, stop=False, skip_group_check=True,
            )
            dr_gate(mt1_ps, htT16, vbt8_sb, False, True)
            mtv_t = small.tile([BL, H], F32, tag="mtv_t")
            nc.scalar.activation(mtv_t[:], mt1_ps[:], ACT.Tanh, scale=1.0 / SG)
            lgt_t = small.tile([BL, 1], F32, tag="lgt_t")
            scr8t = small.tile([BL, H], F32, tag="scr8t")
            nc.vector.scalar_tensor_tensor(
                out=scr8t[:], in0=mtv_t[:], scalar=1.0, in1=wbB_sb[:],
                op0=ALU.mult, op1=ALU.mult, accum_out=lgt_t[:],
            )
            ht2_ps = psG.tile([BL, H], F32, tag="psG", name="ht2")
            dr_gate(ht2_ps, htT16, wqe8_sb, True, True)
            ht2_sb = small.tile([BL, H], F32, tag="ht2_sb")
            nc.vector.scalar_tensor_tensor(
                out=ht2_sb[:], in0=ht2_ps[:], scalar=1.0, in1=bqeB,
                op0=ALU.mult, op1=ALU.add,
            )

            chain3_i(1)

            # ---------- frames gates + logits ----------
            rv8 = small.tile([BL, 1], F32, tag="rv8")
            nc.vector.reciprocal(rv8[:], sum8[:])
            nc.vector.tensor_scalar_mul(hv16n[:], hv16[:], rv8[:])
            for jt in range(KT):
                tp = psX.tile([128, BL], BF16, tag="psX", name=f"hvtp{jt}")
                nc.tensor.transpose(
                    tp[:], hv16n[0:BL, jt * 128:(jt + 1) * 128],
                    eye_sb[0:BL, 0:BL],
                )
                nc.vector.tensor_copy(hvT16[:, jt, 0:BL], tp[:])

            mv1_ps = psG.tile([BL, H], F32, tag="psG", name="mv1")
            nc.tensor.matmul(
                mv1_ps[:], eye_sb[0:BL, 0:BL], wbsv_sb[:],
                start=True, stop=False, skip_group_check=True,
            )
            dr_gate(mv1_ps, hvT16, vbv8_sb, False, True)
            mtv_v = small.tile([BL, H], F32, tag="mtv_v")
            nc.scalar.activation(mtv_v[:], mv1_ps[:], ACT.Tanh, scale=1.0 / SG)
            lgv_t = small.tile([BL, 1], F32, tag="lgv_t")
            scr8v = small.tile([BL, H], F32, tag="scr8v")
            nc.vector.scalar_tensor_tensor(
                out=scr8v[:], in0=mtv_v[:], scalar=1.0, in1=wbB_sb[:],
                op0=ALU.mult, op1=ALU.mult, accum_out=lgv_t[:],
            )

            # ---------- pre-reduced AllGather payload ----------
            # Each core ships (Zc, e0*m, e1*m): Zc = sum of its 16 logit
            # exps, e0/e1 masked to core 0. Post-AG work shrinks to one
            # fp32 matmul against a row selector (no exp / transpose on
            # the critical tail).
            ev = small.tile([BL, 1], BF16, tag="ev")
            et = small.tile([BL, 1], BF16, tag="et")
            nc.scalar.activation(ev[:], lgv_t[:], ACT.Exp)
            nc.scalar.activation(et[:], lgt_t[:], ACT.Exp)
            pay_ps = psE.tile([3, 1], F32, tag="psE", name="payps")
            nc.tensor.matmul(
                pay_ps[:], sel3_sb[0:BL, 0:3], ev[0:BL, 0:1],
                start=True, stop=False, skip_group_check=True,
            )
            nc.tensor.matmul(
                pay_ps[:], sel3_sb[0:BL, 3:6], et[0:BL, 0:1],
                start=False, stop=True, skip_group_check=True,
            )
            payload = small.tile([4, 1], F32, tag="payload")
            nc.vector.memset(payload[:], 0.0)
            nc.vector.tensor_copy(payload[0:3, 0:1], pay_ps[:, :])
            cc_in = dram.tile([4, 1], F32, tag="ccin")
            cc_out = dram.tile([4 * NC, 1], F32, tag="ccout",
                               addr_space="Shared")
            nc.sync.dma_start(cc_in[0:4], payload[:])
            nc.gpsimd.collective_compute(
                "AllGather", ALU.bypass,
                replica_groups=[list(range(NC))],
                ins=[cc_in[:].opt()],
                outs=[cc_out[:].opt()],
            )

            # overlap the AG: hv2 = 256*(hv@Wve.T + bve)
            hv2_ps = psG.tile([BL, H], F32, tag="psG", name="hv2")
            dr_gate(hv2_ps, hvT16, wve8_sb, True, True)
            hv2_sb = small.tile([BL, H], F32, tag="hv2_sb")
            nc.vector.scalar_tensor_tensor(
                out=hv2_sb[:], in0=hv2_ps[:], scalar=1.0, in1=bveB,
                op0=ALU.mult, op1=ALU.add,
            )

            # ---------- global beta from the gathered stats ----------
            g_col = small.tile([4 * NC, 1], F32, tag="g")
            nc.sync.dma_start(g_col[:, :], cc_out[:, :])
            zrow_ps = psB.tile([1, 3], F32, tag="psB", name="zrow")
            nc.tensor.matmul(   # [Z, e0, e1] row = g_col.T @ sel32
                zrow_ps[:], g_col[0:4 * NC, 0:1], sel32_sb[0:4 * NC, 0:3],
                start=True, stop=True, skip_group_check=True,
            )
            rg = small.tile([1, 1], F32, tag="rg")
            nc.vector.reciprocal(rg[:], zrow_ps[:, 0:1])
            betas = small.tile([1, 2], BF16, tag="betas")
            nc.vector.tensor_scalar_mul(betas[:], zrow_ps[:, 1:3], rg[:])
            beta8_ps = psB.tile([BL, 2], F32, tag="psB", name="beta8")
            nc.tensor.matmul(   # beta/256 broadcast to 8 partitions
                beta8_ps[:], oq_sb[0:1, 0:BL], betas[0:1, 0:2],
                start=True, stop=True,
            )
            # ---------- out = tanh(hWhh+bh + b0*hv2 + b1*ht2) ----------
            t1 = small.tile([BL, H], F32, tag="t1")
            nc.vector.scalar_tensor_tensor(
                out=t1[:], in0=hv2_sb[:], scalar=beta8_ps[:, 0:1], in1=hwhh_sb[:],
                op0=ALU.mult, op1=ALU.add,
            )
            s1 = small.tile([BL, H], F32, tag="s1")
            nc.vector.scalar_tensor_tensor(
                out=s1[:], in0=ht2_sb[:], scalar=beta8_ps[:, 1:2], in1=t1[:],
                op0=ALU.mult, op1=ALU.add,
            )
            out_sb = small.tile([BL, H], F32, tag="out_sb")
            nc.scalar.activation(out_sb[:], s1[:], ACT.Tanh)
            nc.sync.dma_start(out_ext, out_sb[:])

    nc.compile()
    return nc


_cached_nc = None


def _get_nc():
    global _cached_nc
    if _cached_nc is None:
        _cached_nc = build_nc()
    return _cached_nc


def _bf(a):
    return np.asarray(a, np.float32).astype(NP_BF16)


def _f8(a):
    return np.asarray(a, np.float32).astype(NP_FP8)


def _pack_w(w):
    """[512,512] -> [128, 4*512] with free = kt*512 + j, k = kt*128 + p."""
    return np.ascontiguousarray(
        np.asarray(w, np.float32).reshape(KT, 128, H).transpose(1, 0, 2)
        .reshape(128, KT * H)
    )


def make_in_maps(inputs):
    h = np.asarray(inputs["h"], np.float32)
    frames = np.asarray(inputs["hidden_frames"], np.float32)
    text = np.asarray(inputs["hidden_text"], np.float32)

    Vav = np.asarray(inputs["Vav"], np.float32)
    Vat = np.asarray(inputs["Vat"], np.float32)
    wb = np.asarray(inputs["wb"], np.float32)

    # vavZ8[p, jtp, q, 2v+c] = 16*Vav[(2jtp+q)*128+p] if c == v (pad to 16)
    vavZ = np.zeros((128, 2, 2, 16), np.float32)
    for jtp in range(2):
        for q in range(2):
            col = Vav[(2 * jtp + q) * 128:(2 * jtp + q + 1) * 128] * SW
            for v in range(2):
                vavZ[:, jtp, q, 2 * v + v] = col

    biasr = np.zeros((1, 8 * H), np.float32)
    biasr = _bf(biasr)
    biasB = np.zeros((BL, 7 * H), np.float32)
    scales = [SW, SW, SG, SG, SG, SG, 1.0]
    for i, (k, s) in enumerate(zip(
            ["bav", "bat", "bve", "bqe", "bbv", "bbt", "bh"], scales)):
        biasB[:, i * H:(i + 1) * H] = np.asarray(inputs[k], np.float32)[None, :] * s
    biasB = _bf(biasB)
    wbB = np.ascontiguousarray(np.broadcast_to(wb, (BL, H))).astype(np.float32)
    blkI = np.zeros((BL, BL, Tt), np.float32)
    for b in range(BL):
        blkI[b, b, :] = 1.0
    blkI = _bf(blkI.reshape(BL, BL * Tt))

    hTp = np.zeros((128, KT, 16), np.float32)   # per-core filled below
    f8e_shared = np.zeros((128, 128 + KT * H), np.float32)
    f8e_shared[:, 64:128] = vavZ.reshape(128, 64)
    f8e_shared[:, 128:] = _pack_w(inputs["Uav"]) * SW
    wav8 = _f8(_pack_w(inputs["Wav"]) * SW)
    uat8 = _f8(_pack_w(inputs["Uat"]) * SW)

    f8t = np.zeros((128, 2 * KT * H), np.float32)
    # tT8 filled per-core below
    f8t[:, KT * H:] = _pack_w(inputs["Wat"]) * SW

    f8g = np.zeros((128, 5 * KT * H), np.float32)
    f8g[:, 0:KT * H] = _pack_w(inputs["Wb"]) * SG
    f8g[:, KT * H:2 * KT * H] = _pack_w(inputs["Vbt"]) * SW
    f8g[:, 2 * KT * H:3 * KT * H] = _pack_w(np.asarray(inputs["Wqe"], np.float32).T) * SW
    f8g[:, 3 * KT * H:4 * KT * H] = _pack_w(inputs["Vbv"]) * SW
    f8g[:, 4 * KT * H:] = _pack_w(np.asarray(inputs["Wve"], np.float32).T) * SW
    f8g = _f8(f8g)

    sel32 = np.zeros((4 * NC, 3), np.float32)
    for k in range(NC):
        sel32[4 * k + 0, 0] = 1.0
        sel32[4 * k + 1, 1] = 1.0
        sel32[4 * k + 2, 2] = 1.0
    shared = dict(
        sel32=sel32,
        f8g=f8g, wav8=wav8, uat8=uat8,
        whh_bf=_bf(_pack_w(inputs["Whh"])),
        biasr=biasr, biasB=biasB, wbB=wbB, blkI=blkI,
    )

    in_maps = []
    for i in range(NC):
        sl = slice(i * BL, (i + 1) * BL)
        fTc = np.ascontiguousarray(
            frames[sl].transpose(0, 2, 1)       # [BL, H, Tv]
            .reshape(BL, KT, 128, Tv)
            .transpose(0, 2, 1, 3)              # [BL, 128, KT, Tv]
            .reshape(BL, 128, KT * Tv)
        )
        fRc = np.ascontiguousarray(
            frames[sl]                          # [BL, Tv, H]
            .reshape(BL, KT, 128, H)
            .transpose(0, 2, 1, 3)              # [BL, 128, TT, H]
            .reshape(BL, 128, KT * H)
        )
        tTc = np.ascontiguousarray(
            text[sl].transpose(2, 0, 1)         # [H, BL, Tt]
            .reshape(KT, 128, BL, Tt)
            .transpose(1, 0, 2, 3)              # [128, KT, BL, Tt]
            .reshape(128, KT * BL * Tt)
        )
        hTc = np.ascontiguousarray(
            h[sl].T.reshape(KT, 128, BL).transpose(1, 0, 2).reshape(128, KT * BL)
        )
        f8e = f8e_shared.copy()
        f8e[:, 0:64] = 0.0
        f8e[:, 0:64].reshape(128, KT, 16)[:, :, 0:BL] = hTc.reshape(128, KT, BL)
        f8t_i = f8t.copy()
        f8t_i[:, 0:KT * H] = tTc
        bfs = np.zeros((128, 128 + KT * BL + KT), np.float32)
        bfs[:, 0:128] = np.eye(128, dtype=np.float32)
        bfs[:, 128:128 + KT * BL] = hTc
        bfs[:, 128 + KT * BL:] = np.ascontiguousarray(
            np.asarray(inputs["Vat"], np.float32).reshape(KT, 128).T)
        sel3 = np.zeros((BL, 6), np.float32)
        sel3[:, 0] = 1.0
        sel3[:, 3] = 1.0
        if i == 0:
            sel3[0, 1] = 1.0
            sel3[1, 2] = 1.0
        in_maps.append(dict(
            shared,
            fT8=_f8(fTc), fR8=_f8(fRc),
            f8e=_f8(f8e), f8t=_f8(f8t_i), bfs=_bf(bfs),
            sel3=_bf(sel3),
        ))
    return in_maps


def run(inputs, trace=False, **kw):
    nc = _get_nc()
    in_maps = make_in_maps(inputs)
    res = run_bass_kernel_spmd(nc, in_maps, core_ids=list(range(NC)), trace=trace, **kw)
    out = np.concatenate([res.results[i]["out"] for i in range(NC)], axis=0)
    return out, res


def kernel(**inputs) -> np.ndarray:
    out, _ = run(inputs, trace=False)
    return out

